# revision 6
# baseline (speedup 1.0000x reference)
# Trainium2 Bass kernel for Mistral-style sliding-window GQA attention.
#
# Problem: hidden [2,1024,4096], 32 q-heads / 8 kv-heads, head_dim 128,
# RoPE (neox), causal + sliding-window(512) attention, out proj.
#
# Sharding: tensor-parallel over heads across 8 cores. Core c owns q-heads
# [4c..4c+3] and kv-head c (wq cols 512c:512c+512, wk/wv cols 128c:+128).
# Each core computes its heads' attention output in TRANSPOSED layout
# [feat, tok]; per-(batch, token-half) AllGathers over the 8 cores
# concatenate the feature (partition) axis to give the full [4096, 512]
# attn output of that half on every core, and each core then applies its
# column shard of wo ([4096, 512]) to produce out[:, 512c:512c+512]. The
# host concatenates the 8 column shards.
#
# v2 schedule: a single software pipeline over the four 512-token phases
# n=0..3 (batch b=n//2, half h=n%2):
#   A(0) B(0,h0) [AG00] A(1) B(0,h1) [AG01] A(2) B(1,h0) [AG10] D(0,0)
#   A(3) B(1,h1) [AG11] D(0,1) D(1,0) D(1,1)
# Attention (phase B) is HALF-MAJOR: all 4 heads complete a 512-token half
# before any head starts the next half, so each AllGather triggers as early
# as possible and hides under >=40us of PE work. All psum tiles are single
# banks ([128,512]), which leaves room for a 3-deep score pipeline (PE
# never waits on ACT's exp).
#
# All matmuls run in bf16 (fp32 PSUM accumulation); softmax math in fp32.
#
# Layout trick: everything is computed transposed ([feature, token]) so that
# every matmul's contraction operand is already partition-major:
#   QT = wq.T @ hid     via matmul(lhsT=wq_chunk,  rhs=hidT_chunk)
#   KT = wk.T @ hid     via matmul(lhsT=wk_chunk,  rhs=hidT_chunk)
#   VT = wv.T @ hid     via matmul(lhsT=wv_chunk,  rhs=hidT_chunk)
#   V  = VT.T           via PE transposes (V needed k-major for O^T)
#   ST = K_j^T Q        via matmul(lhsT=KT_j,      rhs=QT_piece)  [k, q]
#   l  = 1^T A          via matmul(lhsT=ones_col,  rhs=at_piece)  [1, q]
#   OT = V_j^T A        via matmul(lhsT=V_j,       rhs=at_piece)  [d, q]
#   out= ag^T @ wo      via matmul(lhsT=ag_chunk,  rhs=wo_chunk)  [tok, oc]
# Softmax over k (partition axis of ST) uses exp with 0/1 post-multiplies
# for the causal diagonal / window edge, a ones-column matmul for the
# denominator, and recip + partition_broadcast + multiply to normalize.
# RoPE runs entirely on the DVE reading straight from PSUM (no ACT
# staging), so ACT only does exps and small copies.
#
# Queue discipline: DMAs traced after an AllGather trigger on the gpsimd
# queue wait for that collective, so every phase-D ag read is traced on
# gpsimd BEFORE the first trigger of any collective it must not wait for.
# hid/weight loads ride sync/scalar/vector; out writes ride sync.

import functools

import numpy as np
import ml_dtypes

BF16 = ml_dtypes.bfloat16

B, S, HID = 2, 1024, 4096
T = B * S                     # 2048 flattened tokens
NCORES = 8
D = 128                       # head dim
QH = 4                        # q heads per core
QF = QH * D                   # 512 q features per core
HC = HID // 128               # 32 hidden-dim chunks
NJ = S // 128                 # 8 k-tiles per batch
WINDOW = 512
SH = 512                      # tokens per AllGather half
OUTC = HID // NCORES          # 512 out columns per core
SCALE = D ** -0.5

# AllGather output row-block permutation: ag block ci holds the
# contribution (core, local-head) = AG_PERM[ci] (512KB inputs gather
# core-major, one chunk per core).
AG_PERM = [(ci // 4, ci % 4) for ci in range(HC)]

# Half-major score pieces. Phase B(b,h) covers q tokens
# [b*S + 512h, +512). Piece (h, j) is the part of k-tile j's 640-wide
# q-span inside this half: bank columns [c0, c0+w). Pieces with 'diag'
# carry the causal-diagonal 0/1 mask at piece cols [0,128); 'edge' pieces
# carry the window-edge mask at the last 128 cols.
def _piece(h, j):
    if h == 0:
        c0, w = 128 * j, 512 - 128 * j
        diag, edge = True, False
    elif j <= 3:
        c0, w = 0, 128 * (j + 1)
        diag, edge = False, True
    else:
        c0 = 128 * (j - 4)
        w = 512 - c0
        diag, edge = True, False
    return c0, w, diag, edge

H0_JS = [0, 1, 2, 3]              # j=0 piece is full-bank -> leads
H1_JS = [3, 4, 0, 1, 2, 5, 6, 7]  # j=3 piece is full-bank -> leads


def _build():
    import concourse.mybir as mybir
    import concourse.tile as tile
    from concourse import bacc
    from concourse.masks import make_identity

    f32, bf16 = mybir.dt.float32, mybir.dt.bfloat16
    AF = mybir.ActivationFunctionType
    ALU = mybir.AluOpType

    nc = bacc.Bacc(
        "TRN2", target_bir_lowering=False, debug=False, num_devices=NCORES
    )

    hidT = nc.dram_tensor("hidT", [128, HC * T], bf16, kind="ExternalInput")
    wq = nc.dram_tensor("wq", [128, HC * QF], bf16, kind="ExternalInput")
    wk = nc.dram_tensor("wk", [128, HC * D], bf16, kind="ExternalInput")
    wv = nc.dram_tensor("wv", [128, HC * D], bf16, kind="ExternalInput")
    wo = nc.dram_tensor("wo", [128, HC * OUTC], bf16, kind="ExternalInput")
    cosT = nc.dram_tensor("cosT", [128, T], bf16, kind="ExternalInput")
    sinT = nc.dram_tensor("sinT", [128, T], bf16, kind="ExternalInput")
    maskT = nc.dram_tensor("maskT", [128, 256], bf16, kind="ExternalInput")
    out = nc.dram_tensor("out", [T, OUTC], bf16, kind="ExternalOutput")

    with tile.TileContext(nc) as tc:
        with (
            tc.tile_pool(name="ps", bufs=8, space="PSUM") as psp,
            tc.tile_pool(name="consts", bufs=1) as sbp,
            tc.tile_pool(name="hidp", bufs=28) as hidp,
            tc.tile_pool(name="ropep", bufs=2) as ropep,
            tc.tile_pool(name="attnp", bufs=5) as attnp,
            tc.tile_pool(name="miscp", bufs=2) as miscp,
            tc.tile_pool(name="agp", bufs=4) as agp,
            tc.tile_pool(name="dram", bufs=1, space="DRAM") as dramp,
        ):
            def bank(name):
                return psp.tile([128, 512], f32, tag="bank", name=name)

            def load_split(dst, src, ncols, pieces, skip_first=0, eng=None):
                eng = eng or nc.sync
                step = ncols // pieces
                for i in range(skip_first, pieces):
                    eng.dma_start(
                        dst[:, i * step:(i + 1) * step],
                        src[:, i * step:(i + 1) * step],
                    )

            hidT_r = hidT[:, :].rearrange("p (h t) -> p h t", h=HC)

            def load_hid(n, h2, eng):
                # one DMA covers h-chunks 2*h2 and 2*h2+1
                ht = hidp.tile([128, 1024], bf16, tag="hid", bufs=28,
                               name=f"ht{n}_{h2}")
                eng.dma_start(
                    ht[:].rearrange("p (a t) -> p a t", a=2),
                    hidT_r[:, 2 * h2:2 * h2 + 2, n * 512:(n + 1) * 512],
                )
                return ht

            def prefetch_hid(n, eng, eng2=None):
                # eng2 (if given) takes odd tiles so two queues share the
                # startup issue latency
                return [
                    load_hid(n, h2, eng2 if (eng2 and h2 % 2) else eng)
                    for h2 in range(16)
                ]

            # startup-critical loads first: wq/wk/wv h=0..3 chunks + first
            # hid pairs feed the very first matmuls.
            wq_sb = sbp.tile([128, HC * QF], bf16, name="wq_sb")
            wk_sb = sbp.tile([128, HC * D], bf16, name="wk_sb")
            wv_sb = sbp.tile([128, HC * D], bf16, name="wv_sb")
            nc.sync.dma_start(wq_sb[:, 0:1024], wq[:, 0:1024])
            nc.sync.dma_start(wq_sb[:, 1024:2048], wq[:, 1024:2048])
            nc.sync.dma_start(wk_sb[:, 0:1024], wk[:, 0:1024])
            nc.sync.dma_start(wv_sb[:, 0:1024], wv[:, 0:1024])
            pre0 = prefetch_hid(0, nc.scalar, nc.gpsimd)
            load_split(wq_sb, wq, HC * QF, 16, skip_first=2)
            load_split(wk_sb, wk, HC * D, 4, skip_first=1)
            load_split(wv_sb, wv, HC * D, 4, skip_first=1)
            cos_sb = sbp.tile([128, T], bf16, name="cos_sb")
            sin_sb = sbp.tile([128, T], bf16, name="sin_sb")

            # per-batch transposed activations (region-reused across batches)
            QT_sb = sbp.tile([128, QH * S], bf16, name="QT_sb")
            KT_sb = sbp.tile([128, S], bf16, name="KT_sb")
            VT_sb = sbp.tile([128, S], bf16, name="VT_sb")
            V_sb = sbp.tile([128, S], bf16, name="V_sb")

            # small constants for B (tiny DMAs / on-chip init)
            mask_sb = sbp.tile([128, 256], bf16, name="mask_sb")
            nc.sync.dma_start(mask_sb[:], maskT[:, :])
            ones_sb = sbp.tile([128, 1], bf16, name="ones_sb")
            nc.vector.memset(ones_sb[:], 1.0)
            ident_sb = sbp.tile([128, 128], bf16, name="ident_sb")
            make_identity(nc, ident_sb[:])
            wo_sb = sbp.tile([128, HC * OUTC], bf16, name="wo_sb")

            # per-(batch, half) AllGathers
            attn_local = [
                [dramp.tile([QF, SH], bf16, name=f"attn_local{b}_{h}")
                 for h in range(2)]
                for b in range(B)
            ]
            ag_out = [
                [dramp.tile([HID, SH], bf16, name=f"ag_out{b}_{h}",
                            addr_space="Shared")
                 for h in range(2)]
                for b in range(B)
            ]

            def load_trig(n):
                nc.gpsimd.dma_start(
                    cos_sb[:, n * 512:(n + 1) * 512],
                    cosT[:, n * 512:(n + 1) * 512],
                )
                nc.gpsimd.dma_start(
                    sin_sb[:, n * 512:(n + 1) * 512],
                    sinT[:, n * 512:(n + 1) * 512],
                )

            def rope_dve(dst, src_ps, n, tag):
                # neox rotate-half straight from psum on the DVE:
                #   dst = x*cos + swap_halves(x)*sin_signed
                # (sin rows 0:64 arrive pre-negated from the host)
                c = cos_sb[:, n * 512:(n + 1) * 512]
                sg = sin_sb[:, n * 512:(n + 1) * 512]
                t1 = ropep.tile([128, 512], f32, tag="rt1", name=f"t1{tag}")
                t2 = ropep.tile([128, 512], f32, tag="rt2", name=f"t2{tag}")
                nc.vector.tensor_tensor(t1[:], src_ps, c, ALU.mult)
                nc.vector.tensor_tensor(
                    t2[0:64, :], src_ps[64:128, :], sg[0:64, :], ALU.mult
                )
                nc.vector.tensor_tensor(
                    t2[64:128, :], src_ps[0:64, :], sg[64:128, :], ALU.mult
                )
                nc.vector.tensor_tensor(dst, t1[:], t2[:], ALU.add)

            # ---- phase A(n): projections, transposed, weight-stationary ----
            def phase_a(n, pre):
                tok0 = (n % 2) * 512
                q_ps = [bank(f"q{n}_{m}") for m in range(QH)]
                k_ps = bank(f"k{n}")
                v_ps = bank(f"v{n}")
                for hg in range(0, HC, 4):
                    hts = [
                        pre[hg // 2 + k // 2][:, (k % 2) * 512:(k % 2) * 512 + 512]
                        for k in range(4)
                    ]
                    def kv_mms():
                        for k, h in enumerate(range(hg, hg + 4)):
                            nc.tensor.matmul(
                                k_ps[:], wk_sb[:, h * 128:(h + 1) * 128],
                                hts[k], start=(h == 0), stop=(h == HC - 1),
                            )
                        for k, h in enumerate(range(hg, hg + 4)):
                            nc.tensor.matmul(
                                v_ps[:], wv_sb[:, h * 128:(h + 1) * 128],
                                hts[k], start=(h == 0), stop=(h == HC - 1),
                            )
                    def q_mms():
                        for m in range(QH):
                            for k, h in enumerate(range(hg, hg + 4)):
                                nc.tensor.matmul(
                                    q_ps[m][:],
                                    wq_sb[:, (h * QH + m) * 128:(h * QH + m + 1) * 128],
                                    hts[k], start=(h == 0), stop=(h == HC - 1),
                                )
                    if hg == HC - 4:
                        # kv stop early so the K rope overlaps the tail q mms
                        kv_mms()
                        q_mms()
                    else:
                        q_mms()
                        kv_mms()
                rope_dve(KT_sb[:, tok0:tok0 + 512], k_ps[:], n, f"K{n}")
                nc.scalar.copy(VT_sb[:, tok0:tok0 + 512], v_ps[:])
                for m in range(QH):
                    rope_dve(
                        QT_sb[:, m * S + tok0:m * S + tok0 + 512],
                        q_ps[m][:], n, f"q{n}_{m}",
                    )

            # ---- V = VT.T via PE transposes (pads the A->B rope latency) ----
            def v_transpose(n):
                for tt in range(4 * (n % 2), 4 * (n % 2) + 4):
                    trp = psp.tile([128, 128], bf16, tag="bank",
                                   name=f"tr{n}_{tt}")
                    nc.tensor.transpose(
                        trp[:], VT_sb[:, tt * 128:(tt + 1) * 128],
                        ident_sb[:],
                    )
                    nc.vector.tensor_copy(V_sb[:, tt * 128:(tt + 1) * 128],
                                          trp[:])

            # ---- phase B(b,h): half-major windowed attention ----
            def phase_b_half(b, h):
                js = H0_JS if h == 0 else H1_JS
                for m in range(QH):
                    l_ps = bank(f"l{b}{h}{m}")
                    o_ps = bank(f"o{b}{h}{m}")
                    at_tiles = {}

                    def scores(j):
                        c0, w, diag, edge = _piece(h, j)
                        sc = bank(f"sc{b}{h}{m}{j}")
                        kslice = KT_sb[:, j * 128:(j + 1) * 128]
                        q0 = m * S + h * 512 + c0
                        nc.tensor.matmul(
                            sc[:, 0:w], kslice, QT_sb[:, q0:q0 + w],
                            start=True, stop=True,
                        )
                        at = attnp.tile([128, 512], bf16, tag="attn", bufs=5,
                                        name=f"at{b}{h}{m}{j}")
                        nc.scalar.activation(at[:, 0:w], sc[:, 0:w], AF.Exp)
                        if diag:
                            nc.vector.tensor_tensor(
                                at[:, 0:128], at[:, 0:128],
                                mask_sb[:, 0:128], ALU.mult,
                            )
                        if edge:
                            nc.vector.tensor_tensor(
                                at[:, w - 128:w], at[:, w - 128:w],
                                mask_sb[:, 128:256], ALU.mult,
                            )
                        at_tiles[j] = at

                    def acc(j, which, first, final):
                        c0, w, _, _ = _piece(h, j)
                        at = at_tiles[j]
                        if which == "l":
                            nc.tensor.matmul(
                                l_ps[0:1, c0:c0 + w], ones_sb[:],
                                at[:, 0:w], start=first, stop=final,
                            )
                        else:
                            vslice = V_sb[:, j * 128:(j + 1) * 128]
                            nc.tensor.matmul(
                                o_ps[:, c0:c0 + w], vslice,
                                at[:, 0:w], start=first, stop=final,
                            )

                    LOOK = 3
                    for i in range(min(LOOK, len(js))):
                        scores(js[i])
                    for i, j in enumerate(js):
                        if i + LOOK < len(js):
                            scores(js[i + LOOK])
                        acc(j, "l", i == 0, i == len(js) - 1)
                        acc(j, "o", i == 0, i == len(js) - 1)

                    # normalize: oT = o_ps * bcast(1/l), then stage to DRAM
                    l_sb = miscp.tile([1, 512], f32, tag="lsb",
                                      name=f"l_sb{b}{h}{m}")
                    nc.scalar.copy(l_sb[:], l_ps[0:1, :])
                    lrec = miscp.tile([1, 512], f32, tag="lrec",
                                      name=f"lrec{b}{h}{m}")
                    nc.vector.reciprocal_approx_fast(lrec[:], l_sb[:])
                    bcr = miscp.tile([128, 512], f32, tag="bcr",
                                     name=f"bcr{b}{h}{m}")
                    nc.gpsimd.partition_broadcast(bcr[:], lrec[:])
                    oT = miscp.tile([128, 512], bf16, tag="osb",
                                    name=f"oT{b}{h}{m}")
                    nc.vector.tensor_tensor(oT[:], o_ps[:], bcr[:], ALU.mult)
                    nc.gpsimd.dma_start(
                        attn_local[b][h][m * 128:(m + 1) * 128, :], oT[:],
                    )

            def all_gather(b, h):
                nc.gpsimd.collective_compute(
                    "AllGather",
                    ALU.bypass,
                    ins=[attn_local[b][h][:, :]],
                    outs=[ag_out[b][h][:, :]],
                    replica_groups=[list(range(NCORES))],
                )

            # ---- phase D: out projection on this core's wo column shard ----
            # ag reads are traced on gpsimd BEFORE the trigger of any AG they
            # must not wait on (a gpsimd DMA traced after a trigger waits for
            # that collective to complete).
            def phase_d_reads(b, pp):
                ag_ts = []
                for afg in range(0, HC, 4):
                    ag_t = agp.tile([128, 2048], bf16, tag="ag", bufs=4,
                                    name=f"ag{b}_{pp}_{afg}")
                    nc.gpsimd.dma_start(
                        ag_t[:].rearrange("p (a t) -> p a t", a=4),
                        ag_out[b][pp][afg * 128:(afg + 4) * 128, :]
                        .rearrange("(a p) t -> p a t", a=4),
                    )
                    ag_ts.append(ag_t)
                return ag_ts

            def phase_d(b, pp, ag_ts):
                ops = [bank(f"op{b}_{pp}_{q}") for q in range(4)]
                for gi, afg in enumerate(range(0, HC, 4)):
                    ag_t = ag_ts[gi]
                    for tt in range(4):
                        for k, af in enumerate(range(afg, afg + 4)):
                            nc.tensor.matmul(
                                ops[tt][:],
                                ag_t[:, k * 512 + tt * 128:k * 512 + (tt + 1) * 128],
                                wo_sb[:, af * OUTC:(af + 1) * OUTC],
                                start=(af == 0), stop=(af == HC - 1),
                            )
                for q in range(4):
                    ob = miscp.tile([128, 512], bf16, tag="ob",
                                    name=f"ob{b}_{pp}_{q}")
                    # drain psum on alternating engines
                    if q % 2 == 0:
                        nc.vector.tensor_copy(ob[:], ops[q][:])
                    else:
                        nc.scalar.copy(ob[:], ops[q][:])
                    r0 = b * S + pp * 512 + q * 128
                    nc.sync.dma_start(out[r0:r0 + 128, :], ob[:])

            # ---- orchestration ----
            load_trig(0)
            load_trig(1)
            phase_a(0, pre0)
            pre1 = prefetch_hid(1, nc.scalar)
            load_split(wo_sb, wo, HC * OUTC, 16, eng=nc.scalar)
            v_transpose(0)
            phase_b_half(0, 0)
            all_gather(0, 0)
            phase_a(1, pre1)
            pre2 = prefetch_hid(2, nc.scalar)
            load_trig(2)
            v_transpose(1)
            phase_b_half(0, 1)
            all_gather(0, 1)
            phase_a(2, pre2)
            pre3 = prefetch_hid(3, nc.scalar)
            load_trig(3)
            v_transpose(2)
            phase_b_half(1, 0)
            ag00_ts = phase_d_reads(0, 0)   # waits AG00/AG01 (both early)
            all_gather(1, 0)
            phase_d(0, 0, ag00_ts)
            phase_a(3, pre3)
            v_transpose(3)
            phase_b_half(1, 1)
            ag01_ts = phase_d_reads(0, 1)   # gated on AG01 (done)
            ag10_ts = phase_d_reads(1, 0)   # gated on AG10 (done)
            all_gather(1, 1)
            phase_d(0, 1, ag01_ts)
            phase_d(1, 0, ag10_ts)
            ag11_ts = phase_d_reads(1, 1)   # genuinely waits on AG11
            phase_d(1, 1, ag11_ts)

    nc.compile()
    return nc


@functools.lru_cache(maxsize=1)
def _get_nc():
    return _build()


def _prep_in_maps(hidden_states, wq, wk, wv, wo, cos, sin):
    hs = np.ascontiguousarray(np.asarray(hidden_states, np.float32)).reshape(T, HID)
    hidT = hs.T.reshape(HC, 128, T).transpose(1, 0, 2).reshape(128, HC * T)
    hidT = np.ascontiguousarray(hidT).astype(BF16)

    wq = np.asarray(wq, np.float32) * SCALE
    wk = np.asarray(wk, np.float32)
    wv = np.asarray(wv, np.float32)
    wo = np.asarray(wo, np.float32)

    cosT = np.ascontiguousarray(np.asarray(cos, np.float32).T)  # [64, S]
    sinT = np.ascontiguousarray(np.asarray(sin, np.float32).T)
    cosT2 = np.concatenate([cosT, cosT], axis=1)   # [64, T]
    sinT2 = np.concatenate([sinT, sinT], axis=1)
    cos128 = np.concatenate([cosT2, cosT2], axis=0).astype(BF16)  # [128, T]
    sin128 = np.concatenate([-sinT2, sinT2], axis=0).astype(BF16)

    r = np.arange(128)[:, None]
    c = np.arange(128)[None, :]
    SL = np.where(c < r, 0.0, 1.0)  # diag tile: invalid where q < k
    SU = np.where(c > r, 0.0, 1.0)  # window-edge tile: invalid where q-k > W
    maskadd = np.concatenate([SL, SU], axis=1).astype(BF16)

    def shard_w(w, cols, core):
        ws = w[:, core * cols:(core + 1) * cols]
        return np.ascontiguousarray(
            ws.reshape(HC, 128, cols).transpose(1, 0, 2).reshape(128, HC * cols)
        ).astype(BF16)

    def shard_wo(w, core):
        ws = w[:, core * OUTC:(core + 1) * OUTC]
        blocks = []
        for ci in range(HC):
            c2, hp = AG_PERM[ci]
            g = 4 * c2 + hp
            blocks.append(ws[g * 128:(g + 1) * 128, :])
        arr = np.stack(blocks, 0)
        return np.ascontiguousarray(
            arr.transpose(1, 0, 2).reshape(128, HC * OUTC)
        ).astype(BF16)

    in_maps = []
    for cidx in range(NCORES):
        in_maps.append({
            "hidT": hidT,
            "wq": shard_w(wq, QF, cidx),
            "wk": shard_w(wk, D, cidx),
            "wv": shard_w(wv, D, cidx),
            "wo": shard_wo(wo, cidx),
            "cosT": cos128,
            "sinT": sin128,
            "maskT": maskadd,
        })
    return in_maps


def run(inputs, trace=False, **spmd_kwargs):
    from concourse.bass_utils import run_bass_kernel_spmd

    window = int(np.asarray(inputs["window"]))
    assert window == WINDOW, f"kernel compiled for window={WINDOW}, got {window}"
    nc = _get_nc()
    in_maps = _prep_in_maps(
        inputs["hidden_states"], inputs["wq"], inputs["wk"], inputs["wv"],
        inputs["wo"], inputs["cos"], inputs["sin"],
    )
    res = run_bass_kernel_spmd(
        nc, in_maps, list(range(NCORES)), trace=trace, **spmd_kwargs
    )
    parts = [np.asarray(res.results[i]["out"], np.float32) for i in range(NCORES)]
    full = np.concatenate(parts, axis=1).reshape(B, S, HID)
    return full, res


def kernel(**inputs):
    return run(inputs, trace=False)[0]


# revision 17
# speedup vs baseline: 1.0093x; 1.0093x over previous
# Trainium2 Bass kernel for Mistral-style sliding-window GQA attention.
#
# Problem: hidden [2,1024,4096], 32 q-heads / 8 kv-heads, head_dim 128,
# RoPE (neox), causal + sliding-window(512) attention, out proj.
#
# Sharding: tensor-parallel over heads across 8 cores. Core c owns q-heads
# [4c..4c+3] and kv-head c (wq cols 512c:512c+512, wk/wv cols 128c:+128).
# Each core computes its heads' attention output in TRANSPOSED layout
# [feat, tok]; per-(batch, token-half) AllGathers over the 8 cores
# concatenate the feature (partition) axis to give the full [4096, 512]
# attn output of that half on every core, and each core then applies its
# column shard of wo ([4096, 512]) to produce out[:, 512c:512c+512]. The
# host concatenates the 8 column shards.
#
# v2 schedule: a single software pipeline over the four 512-token phases
# n=0..3 (batch b=n//2, half h=n%2):
#   A(0) B(0,h0) [AG00] A(1) B(0,h1) [AG01] A(2) B(1,h0) [AG10] D(0,0)
#   A(3) B(1,h1) [AG11] D(0,1) D(1,0) D(1,1)
# Attention (phase B) is HALF-MAJOR: all 4 heads complete a 512-token half
# before any head starts the next half, so each AllGather triggers as early
# as possible and hides under >=40us of PE work. All psum tiles are single
# banks ([128,512]), which leaves room for a 3-deep score pipeline (PE
# never waits on ACT's exp).
#
# All matmuls run in bf16 (fp32 PSUM accumulation); softmax math in fp32.
#
# Layout trick: everything is computed transposed ([feature, token]) so that
# every matmul's contraction operand is already partition-major:
#   QT = wq.T @ hid     via matmul(lhsT=wq_chunk,  rhs=hidT_chunk)
#   KT = wk.T @ hid     via matmul(lhsT=wk_chunk,  rhs=hidT_chunk)
#   VT = wv.T @ hid     via matmul(lhsT=wv_chunk,  rhs=hidT_chunk)
#   V  = VT.T           via PE transposes (V needed k-major for O^T)
#   ST = K_j^T Q        via matmul(lhsT=KT_j,      rhs=QT_piece)  [k, q]
#   l  = 1^T A          via matmul(lhsT=ones_col,  rhs=at_piece)  [1, q]
#   OT = V_j^T A        via matmul(lhsT=V_j,       rhs=at_piece)  [d, q]
#   out= ag^T @ wo      via matmul(lhsT=ag_chunk,  rhs=wo_chunk)  [tok, oc]
# Softmax over k (partition axis of ST) uses exp with 0/1 post-multiplies
# for the causal diagonal / window edge, a ones-column matmul for the
# denominator, and recip + partition_broadcast + multiply to normalize.
# RoPE runs entirely on the DVE reading straight from PSUM (no ACT
# staging), so ACT only does exps and small copies.
#
# Queue discipline: DMAs traced after an AllGather trigger on the gpsimd
# queue wait for that collective, so every phase-D ag read is traced on
# gpsimd BEFORE the first trigger of any collective it must not wait for.
# hid/weight loads ride sync/scalar/vector; out writes ride sync.

import functools

import numpy as np
import ml_dtypes

BF16 = ml_dtypes.bfloat16

B, S, HID = 2, 1024, 4096
T = B * S                     # 2048 flattened tokens
NCORES = 8
D = 128                       # head dim
QH = 4                        # q heads per core
QF = QH * D                   # 512 q features per core
HC = HID // 128               # 32 hidden-dim chunks
NJ = S // 128                 # 8 k-tiles per batch
WINDOW = 512
SH = 512                      # tokens per AllGather half
OUTC = HID // NCORES          # 512 out columns per core
SCALE = D ** -0.5

# AllGather output row-block permutation: ag block ci holds the
# contribution (core, local-head) = AG_PERM[ci] (512KB inputs gather
# core-major, one chunk per core).
AG_PERM = [(ci // 4, ci % 4) for ci in range(HC)]

# Half-major score pieces. Phase B(b,h) covers q tokens
# [b*S + 512h, +512). Piece (h, j) is the part of k-tile j's 640-wide
# q-span inside this half: bank columns [c0, c0+w). Pieces with 'diag'
# carry the causal-diagonal 0/1 mask at piece cols [0,128); 'edge' pieces
# carry the window-edge mask at the last 128 cols.
def _piece(h, j):
    if h == 0:
        c0, w = 128 * j, 512 - 128 * j
        diag, edge = True, False
    elif j <= 3:
        c0, w = 0, 128 * (j + 1)
        diag, edge = False, True
    else:
        c0 = 128 * (j - 4)
        w = 512 - c0
        diag, edge = True, False
    return c0, w, diag, edge

H0_JS = [0, 1, 2, 3]              # j=0 piece is full-bank -> leads
H1_JS = [3, 4, 0, 1, 2, 5, 6, 7]  # j=3 piece is full-bank -> leads


def _build():
    import concourse.mybir as mybir
    import concourse.tile as tile
    from concourse import bacc
    from concourse.masks import make_identity

    f32, bf16 = mybir.dt.float32, mybir.dt.bfloat16
    AF = mybir.ActivationFunctionType
    ALU = mybir.AluOpType

    nc = bacc.Bacc(
        "TRN2", target_bir_lowering=False, debug=False, num_devices=NCORES
    )

    hidT = nc.dram_tensor("hidT", [128, HC * T], bf16, kind="ExternalInput")
    wq = nc.dram_tensor("wq", [128, HC * QF], bf16, kind="ExternalInput")
    wk = nc.dram_tensor("wk", [128, HC * D], bf16, kind="ExternalInput")
    wv = nc.dram_tensor("wv", [128, HC * D], bf16, kind="ExternalInput")
    wo = nc.dram_tensor("wo", [128, HC * OUTC], bf16, kind="ExternalInput")
    cosT = nc.dram_tensor("cosT", [128, T], bf16, kind="ExternalInput")
    sinT = nc.dram_tensor("sinT", [128, T], bf16, kind="ExternalInput")
    maskT = nc.dram_tensor("maskT", [128, 256], bf16, kind="ExternalInput")
    out = nc.dram_tensor("out", [T, OUTC], bf16, kind="ExternalOutput")

    with tile.TileContext(nc) as tc:
        with (
            # PSUM split by tile lifetime so ring-slot reuse can never put a
            # short-lived tile on a slot whose release is traced behind the
            # blocked PE queue: qp holds the deferred-rope q banks (released
            # mid-B), bb holds l/o accumulators, rp everything short-lived.
            tc.tile_pool(name="qp", bufs=3, space="PSUM") as qpp,
            tc.tile_pool(name="bb", bufs=2, space="PSUM") as bbp,
            tc.tile_pool(name="rp", bufs=3, space="PSUM") as rpp,
            tc.tile_pool(name="consts", bufs=1) as sbp,
            tc.tile_pool(name="hidp", bufs=28) as hidp,
            tc.tile_pool(name="ropep", bufs=2) as ropep,
            tc.tile_pool(name="attnp", bufs=5) as attnp,
            tc.tile_pool(name="miscp", bufs=2) as miscp,
            tc.tile_pool(name="agp", bufs=4) as agp,
            tc.tile_pool(name="dram", bufs=1, space="DRAM") as dramp,
        ):
            def qbank(name):
                return qpp.tile([128, 512], f32, tag="qb", bufs=3, name=name)

            def bbank(name):
                return bbp.tile([128, 512], f32, tag="bb", bufs=2, name=name)

            def rbank(name):
                return rpp.tile([128, 512], f32, tag="rb", bufs=3, name=name)

            def load_split(dst, src, ncols, pieces, skip_first=0, eng=None):
                eng = eng or nc.sync
                step = ncols // pieces
                for i in range(skip_first, pieces):
                    eng.dma_start(
                        dst[:, i * step:(i + 1) * step],
                        src[:, i * step:(i + 1) * step],
                    )

            hidT_r = hidT[:, :].rearrange("p (h t) -> p h t", h=HC)

            def load_hid(n, h2, eng):
                # one DMA covers h-chunks 2*h2 and 2*h2+1
                ht = hidp.tile([128, 1024], bf16, tag="hid", bufs=28,
                               name=f"ht{n}_{h2}")
                eng.dma_start(
                    ht[:].rearrange("p (a t) -> p a t", a=2),
                    hidT_r[:, 2 * h2:2 * h2 + 2, n * 512:(n + 1) * 512],
                )
                return ht

            def prefetch_hid(n, eng, eng2=None):
                # eng2 (if given) takes odd tiles so two queues share the
                # startup issue latency
                return [
                    load_hid(n, h2, eng2 if (eng2 and h2 % 2) else eng)
                    for h2 in range(16)
                ]

            # startup-critical loads first: wq/wk/wv h=0..3 chunks + first
            # hid pairs feed the very first matmuls.
            wq_sb = sbp.tile([128, HC * QF], bf16, name="wq_sb")
            wk_sb = sbp.tile([128, HC * D], bf16, name="wk_sb")
            wv_sb = sbp.tile([128, HC * D], bf16, name="wv_sb")
            nc.sync.dma_start(wq_sb[:, 0:1024], wq[:, 0:1024])
            nc.sync.dma_start(wq_sb[:, 1024:2048], wq[:, 1024:2048])
            nc.sync.dma_start(wk_sb[:, 0:1024], wk[:, 0:1024])
            nc.sync.dma_start(wv_sb[:, 0:1024], wv[:, 0:1024])
            pre0 = prefetch_hid(0, nc.scalar, nc.gpsimd)
            load_split(wq_sb, wq, HC * QF, 16, skip_first=2)
            load_split(wk_sb, wk, HC * D, 4, skip_first=1)
            load_split(wv_sb, wv, HC * D, 4, skip_first=1)
            cos_sb = sbp.tile([128, T], bf16, name="cos_sb")
            sin_sb = sbp.tile([128, T], bf16, name="sin_sb")

            # per-batch transposed activations (region-reused across batches)
            QT_sb = sbp.tile([128, QH * S], bf16, name="QT_sb")
            KT_sb = sbp.tile([128, S], bf16, name="KT_sb")
            VT_sb = sbp.tile([128, S], bf16, name="VT_sb")
            V_sb = sbp.tile([128, S], bf16, name="V_sb")

            # small constants for B (tiny DMAs / on-chip init)
            mask_sb = sbp.tile([128, 256], bf16, name="mask_sb")
            nc.sync.dma_start(mask_sb[:], maskT[:, :])
            ones_sb = sbp.tile([128, 1], bf16, name="ones_sb")
            nc.vector.memset(ones_sb[:], 1.0)
            ident_sb = sbp.tile([128, 128], bf16, name="ident_sb")
            make_identity(nc, ident_sb[:])
            wo_sb = sbp.tile([128, HC * OUTC], bf16, name="wo_sb")

            # per-(batch, half) AllGathers
            attn_local = [
                [dramp.tile([QF, SH], bf16, name=f"attn_local{b}_{h}")
                 for h in range(2)]
                for b in range(B)
            ]
            ag_out = [
                [dramp.tile([HID, SH], bf16, name=f"ag_out{b}_{h}",
                            addr_space="Shared")
                 for h in range(2)]
                for b in range(B)
            ]

            def load_trig(n):
                nc.gpsimd.dma_start(
                    cos_sb[:, n * 512:(n + 1) * 512],
                    cosT[:, n * 512:(n + 1) * 512],
                )
                nc.gpsimd.dma_start(
                    sin_sb[:, n * 512:(n + 1) * 512],
                    sinT[:, n * 512:(n + 1) * 512],
                )

            def rope_dve(dst, src_ps, n, tag):
                # neox rotate-half straight from psum on the DVE:
                #   dst = x*cos + swap_halves(x)*sin_signed
                # (sin rows 0:64 arrive pre-negated from the host)
                c = cos_sb[:, n * 512:(n + 1) * 512]
                sg = sin_sb[:, n * 512:(n + 1) * 512]
                t1 = ropep.tile([128, 512], f32, tag="rt1", name=f"t1{tag}")
                t2 = ropep.tile([128, 512], f32, tag="rt2", name=f"t2{tag}")
                nc.vector.tensor_tensor(t1[:], src_ps, c, ALU.mult)
                nc.vector.tensor_tensor(
                    t2[0:64, :], src_ps[64:128, :], sg[0:64, :], ALU.mult
                )
                nc.vector.tensor_tensor(
                    t2[64:128, :], src_ps[0:64, :], sg[64:128, :], ALU.mult
                )
                nc.vector.tensor_tensor(dst, t1[:], t2[:], ALU.add)

            # ---- phase A(n): projections, transposed, weight-stationary ----
            # Q ropes for heads 1..3 are deferred into phase B (traced just
            # before each head's scores) so B's mask multiplies don't queue
            # on the DVE behind ropes that aren't needed yet.
            def phase_a(n, pre):
                tok0 = (n % 2) * 512
                k_ps = rbank(f"k{n}")
                v_ps = rbank(f"v{n}")
                q_ps = [rbank(f"q{n}_0")] + [
                    qbank(f"q{n}_{m}") for m in range(1, QH)
                ]
                for hg in range(0, HC, 4):
                    hts = [
                        pre[hg // 2 + k // 2][:, (k % 2) * 512:(k % 2) * 512 + 512]
                        for k in range(4)
                    ]
                    def kv_mms():
                        for k, h in enumerate(range(hg, hg + 4)):
                            nc.tensor.matmul(
                                k_ps[:], wk_sb[:, h * 128:(h + 1) * 128],
                                hts[k], start=(h == 0), stop=(h == HC - 1),
                            )
                        for k, h in enumerate(range(hg, hg + 4)):
                            nc.tensor.matmul(
                                v_ps[:], wv_sb[:, h * 128:(h + 1) * 128],
                                hts[k], start=(h == 0), stop=(h == HC - 1),
                            )
                    def q_mms():
                        for m in range(QH):
                            for k, h in enumerate(range(hg, hg + 4)):
                                nc.tensor.matmul(
                                    q_ps[m][:],
                                    wq_sb[:, (h * QH + m) * 128:(h * QH + m + 1) * 128],
                                    hts[k], start=(h == 0), stop=(h == HC - 1),
                                )
                    if hg == HC - 4:
                        # kv stop early so the K rope overlaps the tail q mms
                        kv_mms()
                        q_mms()
                    else:
                        q_mms()
                        kv_mms()
                rope_dve(KT_sb[:, tok0:tok0 + 512], k_ps[:], n, f"K{n}")
                nc.scalar.copy(VT_sb[:, tok0:tok0 + 512], v_ps[:])
                rope_dve(
                    QT_sb[:, 0 * S + tok0:0 * S + tok0 + 512],
                    q_ps[0][:], n, f"q{n}_0",
                )
                return q_ps

            # ---- V = VT.T via PE transposes (pads the A->B rope latency) ----
            def v_transpose(n):
                for tt in range(4 * (n % 2), 4 * (n % 2) + 4):
                    trp = rpp.tile([128, 128], bf16, tag="rb", bufs=3,
                                   name=f"tr{n}_{tt}")
                    nc.tensor.transpose(
                        trp[:], VT_sb[:, tt * 128:(tt + 1) * 128],
                        ident_sb[:],
                    )
                    nc.vector.tensor_copy(V_sb[:, tt * 128:(tt + 1) * 128],
                                          trp[:])

            # ---- phase B(b,h): half-major windowed attention ----
            def phase_b_half(b, h, q_ps):
                n = 2 * b + h
                tok0 = h * 512
                js = H0_JS if h == 0 else H1_JS
                for m in range(QH):
                    if m > 0:
                        rope_dve(
                            QT_sb[:, m * S + tok0:m * S + tok0 + 512],
                            q_ps[m][:], n, f"q{n}_{m}",
                        )
                    l_ps = bbank(f"l{b}{h}{m}")
                    o_ps = bbank(f"o{b}{h}{m}")
                    at_tiles = {}

                    def scores(j):
                        c0, w, diag, edge = _piece(h, j)
                        sc = rbank(f"sc{b}{h}{m}{j}")
                        kslice = KT_sb[:, j * 128:(j + 1) * 128]
                        q0 = m * S + h * 512 + c0
                        nc.tensor.matmul(
                            sc[:, 0:w], kslice, QT_sb[:, q0:q0 + w],
                            start=True, stop=True,
                        )
                        at = attnp.tile([128, 512], bf16, tag="attn", bufs=5,
                                        name=f"at{b}{h}{m}{j}")
                        nc.scalar.activation(at[:, 0:w], sc[:, 0:w], AF.Exp)
                        if diag:
                            nc.vector.tensor_tensor(
                                at[:, 0:128], at[:, 0:128],
                                mask_sb[:, 0:128], ALU.mult,
                            )
                        if edge:
                            nc.vector.tensor_tensor(
                                at[:, w - 128:w], at[:, w - 128:w],
                                mask_sb[:, 128:256], ALU.mult,
                            )
                        at_tiles[j] = at

                    def acc(j, which, first, final):
                        c0, w, _, _ = _piece(h, j)
                        at = at_tiles[j]
                        if which == "l":
                            nc.tensor.matmul(
                                l_ps[0:1, c0:c0 + w], ones_sb[:],
                                at[:, 0:w], start=first, stop=final,
                            )
                        else:
                            vslice = V_sb[:, j * 128:(j + 1) * 128]
                            nc.tensor.matmul(
                                o_ps[:, c0:c0 + w], vslice,
                                at[:, 0:w], start=first, stop=final,
                            )

                    LOOK = 3
                    for i in range(min(LOOK, len(js))):
                        scores(js[i])
                    for i, j in enumerate(js):
                        if i + LOOK < len(js):
                            scores(js[i + LOOK])
                        acc(j, "l", i == 0, i == len(js) - 1)
                        acc(j, "o", i == 0, i == len(js) - 1)

                    # normalize: oT = o_ps * bcast(1/l), then stage to DRAM
                    l_sb = miscp.tile([1, 512], f32, tag="lsb",
                                      name=f"l_sb{b}{h}{m}")
                    nc.scalar.copy(l_sb[:], l_ps[0:1, :])
                    lrec = miscp.tile([1, 512], f32, tag="lrec",
                                      name=f"lrec{b}{h}{m}")
                    nc.vector.reciprocal_approx_fast(lrec[:], l_sb[:])
                    bcr = miscp.tile([128, 512], f32, tag="bcr",
                                     name=f"bcr{b}{h}{m}")
                    nc.gpsimd.partition_broadcast(bcr[:], lrec[:])
                    oT = miscp.tile([128, 512], bf16, tag="osb",
                                    name=f"oT{b}{h}{m}")
                    nc.vector.tensor_tensor(oT[:], o_ps[:], bcr[:], ALU.mult)
                    nc.gpsimd.dma_start(
                        attn_local[b][h][m * 128:(m + 1) * 128, :], oT[:],
                    )

            def all_gather(b, h):
                nc.gpsimd.collective_compute(
                    "AllGather",
                    ALU.bypass,
                    ins=[attn_local[b][h][:, :]],
                    outs=[ag_out[b][h][:, :]],
                    replica_groups=[list(range(NCORES))],
                )

            # ---- phase D: out projection on this core's wo column shard ----
            # ag reads are traced on gpsimd BEFORE the trigger of any AG they
            # must not wait on (a gpsimd DMA traced after a trigger waits for
            # that collective to complete).
            def phase_d_reads(b, pp, eng):
                ag_ts = []
                for afg in range(0, HC, 4):
                    ag_t = agp.tile([128, 2048], bf16, tag="ag", bufs=4,
                                    name=f"ag{b}_{pp}_{afg}")
                    eng.dma_start(
                        ag_t[:].rearrange("p (a t) -> p a t", a=4),
                        ag_out[b][pp][afg * 128:(afg + 4) * 128, :]
                        .rearrange("(a p) t -> p a t", a=4),
                    )
                    ag_ts.append(ag_t)
                return ag_ts

            def phase_d(b, pp, ag_ts):
                # D never overlaps B, so its 4 accumulators borrow the
                # qp (3) + bb (1) banks
                ops = [qbank(f"op{b}_{pp}_{q}") for q in range(3)]
                ops.append(bbank(f"op{b}_{pp}_3"))
                for gi, afg in enumerate(range(0, HC, 4)):
                    ag_t = ag_ts[gi]
                    for tt in range(4):
                        for k, af in enumerate(range(afg, afg + 4)):
                            nc.tensor.matmul(
                                ops[tt][:],
                                ag_t[:, k * 512 + tt * 128:k * 512 + (tt + 1) * 128],
                                wo_sb[:, af * OUTC:(af + 1) * OUTC],
                                start=(af == 0), stop=(af == HC - 1),
                            )
                for q in range(4):
                    ob = miscp.tile([128, 512], bf16, tag="ob",
                                    name=f"ob{b}_{pp}_{q}")
                    # drain psum on alternating engines
                    if q % 2 == 0:
                        nc.vector.tensor_copy(ob[:], ops[q][:])
                    else:
                        nc.scalar.copy(ob[:], ops[q][:])
                    r0 = b * S + pp * 512 + q * 128
                    nc.sync.dma_start(out[r0:r0 + 128, :], ob[:])

            # ---- orchestration ----
            # hid prefetch and wo ride the (otherwise idle) sync queue so
            # the ACT queue stays free for B's exps. Phase-D ag reads also
            # ride sync: the tile framework wires the collective-completion
            # dependency, and the sync queue carries nothing that a blocked
            # read could starve.
            load_trig(0)
            load_trig(1)
            qp0 = phase_a(0, pre0)
            pre1 = prefetch_hid(1, nc.sync)
            load_split(wo_sb, wo, HC * OUTC, 16, eng=nc.sync)
            v_transpose(0)
            phase_b_half(0, 0, qp0)
            all_gather(0, 0)
            qp1 = phase_a(1, pre1)
            pre2 = prefetch_hid(2, nc.sync)
            load_trig(2)
            v_transpose(1)
            phase_b_half(0, 1, qp1)
            all_gather(0, 1)
            qp2 = phase_a(2, pre2)
            pre3 = prefetch_hid(3, nc.sync)
            load_trig(3)
            v_transpose(2)
            phase_b_half(1, 0, qp2)
            all_gather(1, 0)
            ag00_ts = phase_d_reads(0, 0, nc.sync)   # gated on AG00 (done)
            phase_d(0, 0, ag00_ts)
            qp3 = phase_a(3, pre3)
            v_transpose(3)
            phase_b_half(1, 1, qp3)
            all_gather(1, 1)
            ag01_ts = phase_d_reads(0, 1, nc.sync)   # gated on AG01 (done)
            ag10_ts = phase_d_reads(1, 0, nc.sync)   # gated on AG10 (done)
            phase_d(0, 1, ag01_ts)
            phase_d(1, 0, ag10_ts)
            ag11_ts = phase_d_reads(1, 1, nc.sync)   # genuinely waits AG11
            phase_d(1, 1, ag11_ts)

    nc.compile()
    return nc


@functools.lru_cache(maxsize=1)
def _get_nc():
    return _build()


def _prep_in_maps(hidden_states, wq, wk, wv, wo, cos, sin):
    hs = np.ascontiguousarray(np.asarray(hidden_states, np.float32)).reshape(T, HID)
    hidT = hs.T.reshape(HC, 128, T).transpose(1, 0, 2).reshape(128, HC * T)
    hidT = np.ascontiguousarray(hidT).astype(BF16)

    wq = np.asarray(wq, np.float32) * SCALE
    wk = np.asarray(wk, np.float32)
    wv = np.asarray(wv, np.float32)
    wo = np.asarray(wo, np.float32)

    cosT = np.ascontiguousarray(np.asarray(cos, np.float32).T)  # [64, S]
    sinT = np.ascontiguousarray(np.asarray(sin, np.float32).T)
    cosT2 = np.concatenate([cosT, cosT], axis=1)   # [64, T]
    sinT2 = np.concatenate([sinT, sinT], axis=1)
    cos128 = np.concatenate([cosT2, cosT2], axis=0).astype(BF16)  # [128, T]
    sin128 = np.concatenate([-sinT2, sinT2], axis=0).astype(BF16)

    r = np.arange(128)[:, None]
    c = np.arange(128)[None, :]
    SL = np.where(c < r, 0.0, 1.0)  # diag tile: invalid where q < k
    SU = np.where(c > r, 0.0, 1.0)  # window-edge tile: invalid where q-k > W
    maskadd = np.concatenate([SL, SU], axis=1).astype(BF16)

    def shard_w(w, cols, core):
        ws = w[:, core * cols:(core + 1) * cols]
        return np.ascontiguousarray(
            ws.reshape(HC, 128, cols).transpose(1, 0, 2).reshape(128, HC * cols)
        ).astype(BF16)

    def shard_wo(w, core):
        ws = w[:, core * OUTC:(core + 1) * OUTC]
        blocks = []
        for ci in range(HC):
            c2, hp = AG_PERM[ci]
            g = 4 * c2 + hp
            blocks.append(ws[g * 128:(g + 1) * 128, :])
        arr = np.stack(blocks, 0)
        return np.ascontiguousarray(
            arr.transpose(1, 0, 2).reshape(128, HC * OUTC)
        ).astype(BF16)

    in_maps = []
    for cidx in range(NCORES):
        in_maps.append({
            "hidT": hidT,
            "wq": shard_w(wq, QF, cidx),
            "wk": shard_w(wk, D, cidx),
            "wv": shard_w(wv, D, cidx),
            "wo": shard_wo(wo, cidx),
            "cosT": cos128,
            "sinT": sin128,
            "maskT": maskadd,
        })
    return in_maps


def run(inputs, trace=False, **spmd_kwargs):
    from concourse.bass_utils import run_bass_kernel_spmd

    window = int(np.asarray(inputs["window"]))
    assert window == WINDOW, f"kernel compiled for window={WINDOW}, got {window}"
    nc = _get_nc()
    in_maps = _prep_in_maps(
        inputs["hidden_states"], inputs["wq"], inputs["wk"], inputs["wv"],
        inputs["wo"], inputs["cos"], inputs["sin"],
    )
    res = run_bass_kernel_spmd(
        nc, in_maps, list(range(NCORES)), trace=trace, **spmd_kwargs
    )
    parts = [np.asarray(res.results[i]["out"], np.float32) for i in range(NCORES)]
    full = np.concatenate(parts, axis=1).reshape(B, S, HID)
    return full, res


def kernel(**inputs):
    return run(inputs, trace=False)[0]


# revision 22
# speedup vs baseline: 1.0119x; 1.0026x over previous
# Trainium2 Bass kernel for Mistral-style sliding-window GQA attention.
#
# Problem: hidden [2,1024,4096], 32 q-heads / 8 kv-heads, head_dim 128,
# RoPE (neox), causal + sliding-window(512) attention, out proj.
#
# Sharding: tensor-parallel over heads across 8 cores. Core c owns q-heads
# [4c..4c+3] and kv-head c (wq cols 512c:512c+512, wk/wv cols 128c:+128).
# Each core computes its heads' attention output in TRANSPOSED layout
# [feat, tok]; per-(batch, token-half) AllGathers over the 8 cores
# concatenate the feature (partition) axis to give the full [4096, 512]
# attn output of that half on every core, and each core then applies its
# column shard of wo ([4096, 512]) to produce out[:, 512c:512c+512]. The
# host concatenates the 8 column shards.
#
# v2 schedule: a single software pipeline over the four 512-token phases
# n=0..3 (batch b=n//2, half h=n%2):
#   A(0) B(0,h0) [AG00] A(1) B(0,h1) [AG01] A(2) B(1,h0) [AG10] D(0,0)
#   A(3) B(1,h1) [AG11] D(0,1) D(1,0) D(1,1)
# Attention (phase B) is HALF-MAJOR: all 4 heads complete a 512-token half
# before any head starts the next half, so each AllGather triggers as early
# as possible and hides under >=40us of PE work. All psum tiles are single
# banks ([128,512]), which leaves room for a 3-deep score pipeline (PE
# never waits on ACT's exp).
#
# All matmuls run in bf16 (fp32 PSUM accumulation); softmax math in fp32.
#
# Layout trick: everything is computed transposed ([feature, token]) so that
# every matmul's contraction operand is already partition-major:
#   QT = wq.T @ hid     via matmul(lhsT=wq_chunk,  rhs=hidT_chunk)
#   KT = wk.T @ hid     via matmul(lhsT=wk_chunk,  rhs=hidT_chunk)
#   VT = wv.T @ hid     via matmul(lhsT=wv_chunk,  rhs=hidT_chunk)
#   V  = VT.T           via PE transposes (V needed k-major for O^T)
#   ST = K_j^T Q        via matmul(lhsT=KT_j,      rhs=QT_piece)  [k, q]
#   l  = 1^T A          via matmul(lhsT=ones_col,  rhs=at_piece)  [1, q]
#   OT = V_j^T A        via matmul(lhsT=V_j,       rhs=at_piece)  [d, q]
#   out= ag^T @ wo      via matmul(lhsT=ag_chunk,  rhs=wo_chunk)  [tok, oc]
# Softmax over k (partition axis of ST) uses exp with 0/1 post-multiplies
# for the causal diagonal / window edge, a ones-column matmul for the
# denominator, and recip + partition_broadcast + multiply to normalize.
# RoPE runs entirely on the DVE reading straight from PSUM (no ACT
# staging), so ACT only does exps and small copies.
#
# Queue discipline: DMAs traced after an AllGather trigger on the gpsimd
# queue wait for that collective, so every phase-D ag read is traced on
# gpsimd BEFORE the first trigger of any collective it must not wait for.
# hid/weight loads ride sync/scalar/vector; out writes ride sync.

import functools

import numpy as np
import ml_dtypes

BF16 = ml_dtypes.bfloat16

B, S, HID = 2, 1024, 4096
T = B * S                     # 2048 flattened tokens
NCORES = 8
D = 128                       # head dim
QH = 4                        # q heads per core
QF = QH * D                   # 512 q features per core
HC = HID // 128               # 32 hidden-dim chunks
NJ = S // 128                 # 8 k-tiles per batch
WINDOW = 512
SH = 512                      # tokens per AllGather half
OUTC = HID // NCORES          # 512 out columns per core
SCALE = D ** -0.5

# AllGather output row-block permutation: ag block ci holds the
# contribution (core, local-head) = AG_PERM[ci] (512KB inputs gather
# core-major, one chunk per core).
AG_PERM = [(ci // 4, ci % 4) for ci in range(HC)]

# Half-major score pieces. Phase B(b,h) covers q tokens
# [b*S + 512h, +512). Piece (h, j) is the part of k-tile j's 640-wide
# q-span inside this half: bank columns [c0, c0+w). Pieces with 'diag'
# carry the causal-diagonal 0/1 mask at piece cols [0,128); 'edge' pieces
# carry the window-edge mask at the last 128 cols.
def _piece(h, j):
    if h == 0:
        c0, w = 128 * j, 512 - 128 * j
        diag, edge = True, False
    elif j <= 3:
        c0, w = 0, 128 * (j + 1)
        diag, edge = False, True
    else:
        c0 = 128 * (j - 4)
        w = 512 - c0
        diag, edge = True, False
    return c0, w, diag, edge

H0_JS = [0, 1, 2, 3]              # j=0 piece is full-bank -> leads
H1_JS = [3, 4, 0, 1, 2, 5, 6, 7]  # j=3 piece is full-bank -> leads


def _build():
    import concourse.mybir as mybir
    import concourse.tile as tile
    from concourse import bacc
    from concourse.masks import make_identity

    f32, bf16 = mybir.dt.float32, mybir.dt.bfloat16
    AF = mybir.ActivationFunctionType
    ALU = mybir.AluOpType

    nc = bacc.Bacc(
        "TRN2", target_bir_lowering=False, debug=False, num_devices=NCORES
    )

    hidT = nc.dram_tensor("hidT", [128, HC * T], bf16, kind="ExternalInput")
    wq = nc.dram_tensor("wq", [128, HC * QF], bf16, kind="ExternalInput")
    wk = nc.dram_tensor("wk", [128, HC * D], bf16, kind="ExternalInput")
    wv = nc.dram_tensor("wv", [128, HC * D], bf16, kind="ExternalInput")
    wo = nc.dram_tensor("wo", [128, HC * OUTC], bf16, kind="ExternalInput")
    cosT = nc.dram_tensor("cosT", [128, T], bf16, kind="ExternalInput")
    sinT = nc.dram_tensor("sinT", [128, T], bf16, kind="ExternalInput")
    maskT = nc.dram_tensor("maskT", [128, 256], bf16, kind="ExternalInput")
    out = nc.dram_tensor("out", [T, OUTC], bf16, kind="ExternalOutput")

    with tile.TileContext(nc) as tc:
        with (
            # PSUM split by tile lifetime so ring-slot reuse can never put a
            # short-lived tile on a slot whose release is traced behind the
            # blocked PE queue: qp holds the deferred-rope q banks (released
            # mid-B), bb holds l/o accumulators, rp everything short-lived.
            tc.tile_pool(name="qp", bufs=3, space="PSUM") as qpp,
            tc.tile_pool(name="bb", bufs=2, space="PSUM") as bbp,
            tc.tile_pool(name="rp", bufs=3, space="PSUM") as rpp,
            tc.tile_pool(name="consts", bufs=1) as sbp,
            tc.tile_pool(name="hidp", bufs=27) as hidp,
            tc.tile_pool(name="ropep", bufs=2) as ropep,
            tc.tile_pool(name="attnp", bufs=11) as attnp,
            tc.tile_pool(name="miscp", bufs=2) as miscp,
            tc.tile_pool(name="agp", bufs=2) as agp,
            tc.tile_pool(name="dram", bufs=1, space="DRAM") as dramp,
        ):
            def qbank(name):
                return qpp.tile([128, 512], f32, tag="qb", bufs=3, name=name)

            def bbank(name):
                return bbp.tile([128, 512], f32, tag="bb", bufs=2, name=name)

            def rbank(name):
                return rpp.tile([128, 512], f32, tag="rb", bufs=3, name=name)

            def load_split(dst, src, ncols, pieces, skip_first=0, eng=None):
                eng = eng or nc.sync
                step = ncols // pieces
                for i in range(skip_first, pieces):
                    eng.dma_start(
                        dst[:, i * step:(i + 1) * step],
                        src[:, i * step:(i + 1) * step],
                    )

            hidT_r = hidT[:, :].rearrange("p (h t) -> p h t", h=HC)

            def load_hid(n, h2, eng):
                # one DMA covers h-chunks 2*h2 and 2*h2+1
                ht = hidp.tile([128, 1024], bf16, tag="hid", bufs=27,
                               name=f"ht{n}_{h2}")
                eng.dma_start(
                    ht[:].rearrange("p (a t) -> p a t", a=2),
                    hidT_r[:, 2 * h2:2 * h2 + 2, n * 512:(n + 1) * 512],
                )
                return ht

            def prefetch_hid(n, eng, eng2=None):
                # eng2 (if given) takes odd tiles so two queues share the
                # startup issue latency
                return [
                    load_hid(n, h2, eng2 if (eng2 and h2 % 2) else eng)
                    for h2 in range(16)
                ]

            # startup-critical loads first, interleaved in consumption order
            # across sync (even wq pieces + wk/wv) and scalar (hid evens +
            # odd wq pieces); hid odds ride gpsimd. Group g of phase A needs
            # wq pieces 2g/2g+1 and hid tiles 2g/2g+1.
            wq_sb = sbp.tile([128, HC * QF], bf16, name="wq_sb")
            wk_sb = sbp.tile([128, HC * D], bf16, name="wk_sb")
            wv_sb = sbp.tile([128, HC * D], bf16, name="wv_sb")

            def wq_piece(i, eng):
                eng.dma_start(wq_sb[:, i * 1024:(i + 1) * 1024],
                              wq[:, i * 1024:(i + 1) * 1024])

            wq_piece(0, nc.sync)
            wq_piece(1, nc.sync)
            nc.sync.dma_start(wk_sb[:, 0:1024], wk[:, 0:1024])
            nc.sync.dma_start(wv_sb[:, 0:1024], wv[:, 0:1024])
            pre0 = []
            for h2 in range(16):
                if h2 % 2 == 0:
                    pre0.append(load_hid(0, h2, nc.scalar))
                    if h2 >= 2:
                        wq_piece(h2, nc.sync)
                        wq_piece(h2 + 1, nc.scalar)
                else:
                    pre0.append(load_hid(0, h2, nc.gpsimd))
            load_split(wk_sb, wk, HC * D, 4, skip_first=1)
            load_split(wv_sb, wv, HC * D, 4, skip_first=1)
            cos_sb = sbp.tile([128, T], bf16, name="cos_sb")
            sin_sb = sbp.tile([128, T], bf16, name="sin_sb")

            # per-batch transposed activations (region-reused across batches)
            QT_sb = sbp.tile([128, QH * S], bf16, name="QT_sb")
            KT_sb = sbp.tile([128, S], bf16, name="KT_sb")
            VT_sb = sbp.tile([128, S], bf16, name="VT_sb")
            V_sb = sbp.tile([128, S], bf16, name="V_sb")

            # small constants for B (tiny DMAs / on-chip init)
            mask_sb = sbp.tile([128, 256], bf16, name="mask_sb")
            nc.sync.dma_start(mask_sb[:], maskT[:, :])
            ones_sb = sbp.tile([128, 1], bf16, name="ones_sb")
            nc.vector.memset(ones_sb[:], 1.0)
            ident_sb = sbp.tile([128, 128], bf16, name="ident_sb")
            make_identity(nc, ident_sb[:])
            wo_sb = sbp.tile([128, HC * OUTC], bf16, name="wo_sb")

            # per-(batch, half) AllGathers
            attn_local = [
                [dramp.tile([QF, SH], bf16, name=f"attn_local{b}_{h}")
                 for h in range(2)]
                for b in range(B)
            ]
            ag_out = [
                [dramp.tile([HID, SH], bf16, name=f"ag_out{b}_{h}",
                            addr_space="Shared")
                 for h in range(2)]
                for b in range(B)
            ]

            def load_trig(n):
                nc.gpsimd.dma_start(
                    cos_sb[:, n * 512:(n + 1) * 512],
                    cosT[:, n * 512:(n + 1) * 512],
                )
                nc.gpsimd.dma_start(
                    sin_sb[:, n * 512:(n + 1) * 512],
                    sinT[:, n * 512:(n + 1) * 512],
                )

            def rope_dve(dst, src_ps, n, tag):
                # neox rotate-half straight from psum on the DVE:
                #   dst = x*cos + swap_halves(x)*sin_signed
                # (sin rows 0:64 arrive pre-negated from the host)
                c = cos_sb[:, n * 512:(n + 1) * 512]
                sg = sin_sb[:, n * 512:(n + 1) * 512]
                t1 = ropep.tile([128, 512], f32, tag="rt1", name=f"t1{tag}")
                t2 = ropep.tile([128, 512], f32, tag="rt2", name=f"t2{tag}")
                nc.vector.tensor_tensor(t1[:], src_ps, c, ALU.mult)
                nc.vector.tensor_tensor(
                    t2[0:64, :], src_ps[64:128, :], sg[0:64, :], ALU.mult
                )
                nc.vector.tensor_tensor(
                    t2[64:128, :], src_ps[0:64, :], sg[64:128, :], ALU.mult
                )
                nc.vector.tensor_tensor(dst, t1[:], t2[:], ALU.add)

            # ---- phase A(n): projections, transposed, weight-stationary ----
            # Q ropes for heads 1..3 are deferred into phase B (traced just
            # before each head's scores) so B's mask multiplies don't queue
            # on the DVE behind ropes that aren't needed yet.
            def phase_a(n, pre):
                tok0 = (n % 2) * 512
                k_ps = rbank(f"k{n}")
                v_ps = rbank(f"v{n}")
                q_ps = [rbank(f"q{n}_0")] + [
                    qbank(f"q{n}_{m}") for m in range(1, QH)
                ]
                for hg in range(0, HC, 4):
                    hts = [
                        pre[hg // 2 + k // 2][:, (k % 2) * 512:(k % 2) * 512 + 512]
                        for k in range(4)
                    ]
                    def kv_mms():
                        for k, h in enumerate(range(hg, hg + 4)):
                            nc.tensor.matmul(
                                k_ps[:], wk_sb[:, h * 128:(h + 1) * 128],
                                hts[k], start=(h == 0), stop=(h == HC - 1),
                            )
                        for k, h in enumerate(range(hg, hg + 4)):
                            nc.tensor.matmul(
                                v_ps[:], wv_sb[:, h * 128:(h + 1) * 128],
                                hts[k], start=(h == 0), stop=(h == HC - 1),
                            )
                    def q_mms():
                        for m in range(QH):
                            for k, h in enumerate(range(hg, hg + 4)):
                                nc.tensor.matmul(
                                    q_ps[m][:],
                                    wq_sb[:, (h * QH + m) * 128:(h * QH + m + 1) * 128],
                                    hts[k], start=(h == 0), stop=(h == HC - 1),
                                )
                    if hg == HC - 4:
                        # kv stop early so the K rope overlaps the tail q mms
                        kv_mms()
                        q_mms()
                    else:
                        q_mms()
                        kv_mms()
                rope_dve(KT_sb[:, tok0:tok0 + 512], k_ps[:], n, f"K{n}")
                nc.scalar.copy(VT_sb[:, tok0:tok0 + 512], v_ps[:])
                rope_dve(
                    QT_sb[:, 0 * S + tok0:0 * S + tok0 + 512],
                    q_ps[0][:], n, f"q{n}_0",
                )
                return q_ps

            # ---- V = VT.T via PE transposes (pads the A->B rope latency) ----
            def v_transpose(n):
                for tt in range(4 * (n % 2), 4 * (n % 2) + 4):
                    trp = rpp.tile([128, 128], bf16, tag="rb", bufs=3,
                                   name=f"tr{n}_{tt}")
                    nc.tensor.transpose(
                        trp[:], VT_sb[:, tt * 128:(tt + 1) * 128],
                        ident_sb[:],
                    )
                    nc.vector.tensor_copy(V_sb[:, tt * 128:(tt + 1) * 128],
                                          trp[:])

            # ---- phase B(b,h): half-major windowed attention ----
            def phase_b_half(b, h, q_ps):
                n = 2 * b + h
                tok0 = h * 512
                js = H0_JS if h == 0 else H1_JS
                for m in range(QH):
                    if m > 0:
                        rope_dve(
                            QT_sb[:, m * S + tok0:m * S + tok0 + 512],
                            q_ps[m][:], n, f"q{n}_{m}",
                        )
                    l_ps = bbank(f"l{b}{h}{m}")
                    o_ps = bbank(f"o{b}{h}{m}")
                    at_tiles = {}

                    def scores(j):
                        c0, w, diag, edge = _piece(h, j)
                        sc = rbank(f"sc{b}{h}{m}{j}")
                        kslice = KT_sb[:, j * 128:(j + 1) * 128]
                        q0 = m * S + h * 512 + c0
                        nc.tensor.matmul(
                            sc[:, 0:w], kslice, QT_sb[:, q0:q0 + w],
                            start=True, stop=True,
                        )
                        at = attnp.tile([128, 512], bf16, tag="attn", bufs=11,
                                        name=f"at{b}{h}{m}{j}")
                        nc.scalar.activation(at[:, 0:w], sc[:, 0:w], AF.Exp)
                        if diag:
                            nc.vector.tensor_tensor(
                                at[:, 0:128], at[:, 0:128],
                                mask_sb[:, 0:128], ALU.mult,
                            )
                        if edge:
                            nc.vector.tensor_tensor(
                                at[:, w - 128:w], at[:, w - 128:w],
                                mask_sb[:, 128:256], ALU.mult,
                            )
                        at_tiles[j] = at

                    def acc(j, which, first, final):
                        c0, w, _, _ = _piece(h, j)
                        at = at_tiles[j]
                        if which == "l":
                            nc.tensor.matmul(
                                l_ps[0:1, c0:c0 + w], ones_sb[:],
                                at[:, 0:w], start=first, stop=final,
                            )
                        else:
                            vslice = V_sb[:, j * 128:(j + 1) * 128]
                            nc.tensor.matmul(
                                o_ps[:, c0:c0 + w], vslice,
                                at[:, 0:w], start=first, stop=final,
                            )

                    # all l accs run before the o accs so l stops early and
                    # the norm chain (lsb/recip/bcast) overlaps the o pass,
                    # putting the oT write (and the AG trigger it feeds)
                    # right behind the last o matmul
                    LOOK = 3
                    for i in range(min(LOOK, len(js))):
                        scores(js[i])
                    for i, j in enumerate(js):
                        if i + LOOK < len(js):
                            scores(js[i + LOOK])
                        acc(j, "l", i == 0, i == len(js) - 1)
                    for i, j in enumerate(js):
                        acc(j, "o", i == 0, i == len(js) - 1)

                    # normalize: oT = o_ps * bcast(1/l), then stage to DRAM
                    l_sb = miscp.tile([1, 512], f32, tag="lsb",
                                      name=f"l_sb{b}{h}{m}")
                    nc.scalar.copy(l_sb[:], l_ps[0:1, :])
                    lrec = miscp.tile([1, 512], f32, tag="lrec",
                                      name=f"lrec{b}{h}{m}")
                    nc.vector.reciprocal_approx_fast(lrec[:], l_sb[:])
                    bcr = miscp.tile([128, 512], f32, tag="bcr",
                                     name=f"bcr{b}{h}{m}")
                    nc.gpsimd.partition_broadcast(bcr[:], lrec[:])
                    oT = miscp.tile([128, 512], bf16, tag="osb",
                                    name=f"oT{b}{h}{m}")
                    nc.vector.tensor_tensor(oT[:], o_ps[:], bcr[:], ALU.mult)
                    nc.gpsimd.dma_start(
                        attn_local[b][h][m * 128:(m + 1) * 128, :], oT[:],
                    )

            def all_gather(b, h):
                nc.gpsimd.collective_compute(
                    "AllGather",
                    ALU.bypass,
                    ins=[attn_local[b][h][:, :]],
                    outs=[ag_out[b][h][:, :]],
                    replica_groups=[list(range(NCORES))],
                )

            # ---- phase D: out projection on this core's wo column shard ----
            # ag reads are traced on gpsimd BEFORE the trigger of any AG they
            # must not wait on (a gpsimd DMA traced after a trigger waits for
            # that collective to complete).
            def phase_d_reads(b, pp):
                # 4 big reads (8 row-blocks each) keep the gpsimd issue
                # count low so a following AG trigger isn't delayed
                ag_ts = []
                for afg in range(0, HC, 8):
                    ag_t = agp.tile([128, 4096], bf16, tag="ag", bufs=2,
                                    name=f"ag{b}_{pp}_{afg}")
                    nc.gpsimd.dma_start(
                        ag_t[:].rearrange("p (a t) -> p a t", a=8),
                        ag_out[b][pp][afg * 128:(afg + 8) * 128, :]
                        .rearrange("(a p) t -> p a t", a=8),
                    )
                    ag_ts.append(ag_t)
                return ag_ts

            def phase_d(b, pp, ag_ts):
                # D never overlaps B, so its 4 accumulators borrow the
                # qp (3) + bb (1) banks
                ops = [qbank(f"op{b}_{pp}_{q}") for q in range(3)]
                ops.append(bbank(f"op{b}_{pp}_3"))
                for gi, afg in enumerate(range(0, HC, 4)):
                    ag_t = ag_ts[gi // 2]
                    c0 = (gi % 2) * 2048
                    for tt in range(4):
                        for k, af in enumerate(range(afg, afg + 4)):
                            nc.tensor.matmul(
                                ops[tt][:],
                                ag_t[:, c0 + k * 512 + tt * 128:
                                     c0 + k * 512 + (tt + 1) * 128],
                                wo_sb[:, af * OUTC:(af + 1) * OUTC],
                                start=(af == 0), stop=(af == HC - 1),
                            )
                for q in range(4):
                    ob = miscp.tile([128, 512], bf16, tag="ob",
                                    name=f"ob{b}_{pp}_{q}")
                    # drain psum on alternating engines
                    if q % 2 == 0:
                        nc.vector.tensor_copy(ob[:], ops[q][:])
                    else:
                        nc.scalar.copy(ob[:], ops[q][:])
                    r0 = b * S + pp * 512 + q * 128
                    nc.sync.dma_start(out[r0:r0 + 128, :], ob[:])

            # ---- orchestration ----
            # hid prefetch and wo ride the (otherwise idle) sync queue so
            # the ACT queue stays free for B's exps. Phase-D ag reads also
            # ride sync: the tile framework wires the collective-completion
            # dependency, and the sync queue carries nothing that a blocked
            # read could starve.
            load_trig(0)
            load_trig(1)
            qp0 = phase_a(0, pre0)
            pre1 = prefetch_hid(1, nc.sync)
            load_split(wo_sb, wo, HC * OUTC, 16, eng=nc.sync)
            v_transpose(0)
            phase_b_half(0, 0, qp0)
            all_gather(0, 0)
            qp1 = phase_a(1, pre1)
            pre2 = prefetch_hid(2, nc.sync)
            load_trig(2)
            v_transpose(1)
            phase_b_half(0, 1, qp1)
            all_gather(0, 1)
            qp2 = phase_a(2, pre2)
            pre3 = prefetch_hid(3, nc.sync)
            load_trig(3)
            v_transpose(2)
            phase_b_half(1, 0, qp2)
            ag00_ts = phase_d_reads(0, 0)   # gated on AG00/AG01 (both done)
            all_gather(1, 0)
            phase_d(0, 0, ag00_ts)
            qp3 = phase_a(3, pre3)
            v_transpose(3)
            phase_b_half(1, 1, qp3)
            ag01_ts = phase_d_reads(0, 1)   # gated on AG01 (done)
            ag10_ts = phase_d_reads(1, 0)   # gated on AG10 (done)
            all_gather(1, 1)
            phase_d(0, 1, ag01_ts)
            phase_d(1, 0, ag10_ts)
            ag11_ts = phase_d_reads(1, 1)   # genuinely waits on AG11
            phase_d(1, 1, ag11_ts)

    nc.compile()
    return nc


@functools.lru_cache(maxsize=1)
def _get_nc():
    return _build()


def _prep_in_maps(hidden_states, wq, wk, wv, wo, cos, sin):
    hs = np.ascontiguousarray(np.asarray(hidden_states, np.float32)).reshape(T, HID)
    hidT = hs.T.reshape(HC, 128, T).transpose(1, 0, 2).reshape(128, HC * T)
    hidT = np.ascontiguousarray(hidT).astype(BF16)

    wq = np.asarray(wq, np.float32) * SCALE
    wk = np.asarray(wk, np.float32)
    wv = np.asarray(wv, np.float32)
    wo = np.asarray(wo, np.float32)

    cosT = np.ascontiguousarray(np.asarray(cos, np.float32).T)  # [64, S]
    sinT = np.ascontiguousarray(np.asarray(sin, np.float32).T)
    cosT2 = np.concatenate([cosT, cosT], axis=1)   # [64, T]
    sinT2 = np.concatenate([sinT, sinT], axis=1)
    cos128 = np.concatenate([cosT2, cosT2], axis=0).astype(BF16)  # [128, T]
    sin128 = np.concatenate([-sinT2, sinT2], axis=0).astype(BF16)

    r = np.arange(128)[:, None]
    c = np.arange(128)[None, :]
    SL = np.where(c < r, 0.0, 1.0)  # diag tile: invalid where q < k
    SU = np.where(c > r, 0.0, 1.0)  # window-edge tile: invalid where q-k > W
    maskadd = np.concatenate([SL, SU], axis=1).astype(BF16)

    def shard_w(w, cols, core):
        ws = w[:, core * cols:(core + 1) * cols]
        return np.ascontiguousarray(
            ws.reshape(HC, 128, cols).transpose(1, 0, 2).reshape(128, HC * cols)
        ).astype(BF16)

    def shard_wo(w, core):
        ws = w[:, core * OUTC:(core + 1) * OUTC]
        blocks = []
        for ci in range(HC):
            c2, hp = AG_PERM[ci]
            g = 4 * c2 + hp
            blocks.append(ws[g * 128:(g + 1) * 128, :])
        arr = np.stack(blocks, 0)
        return np.ascontiguousarray(
            arr.transpose(1, 0, 2).reshape(128, HC * OUTC)
        ).astype(BF16)

    in_maps = []
    for cidx in range(NCORES):
        in_maps.append({
            "hidT": hidT,
            "wq": shard_w(wq, QF, cidx),
            "wk": shard_w(wk, D, cidx),
            "wv": shard_w(wv, D, cidx),
            "wo": shard_wo(wo, cidx),
            "cosT": cos128,
            "sinT": sin128,
            "maskT": maskadd,
        })
    return in_maps


def run(inputs, trace=False, **spmd_kwargs):
    from concourse.bass_utils import run_bass_kernel_spmd

    window = int(np.asarray(inputs["window"]))
    assert window == WINDOW, f"kernel compiled for window={WINDOW}, got {window}"
    nc = _get_nc()
    in_maps = _prep_in_maps(
        inputs["hidden_states"], inputs["wq"], inputs["wk"], inputs["wv"],
        inputs["wo"], inputs["cos"], inputs["sin"],
    )
    res = run_bass_kernel_spmd(
        nc, in_maps, list(range(NCORES)), trace=trace, **spmd_kwargs
    )
    parts = [np.asarray(res.results[i]["out"], np.float32) for i in range(NCORES)]
    full = np.concatenate(parts, axis=1).reshape(B, S, HID)
    return full, res


def kernel(**inputs):
    return run(inputs, trace=False)[0]


# revision 27
# speedup vs baseline: 1.1311x; 1.1179x over previous
# Trainium2 Bass kernel for Mistral-style sliding-window GQA attention.
#
# Problem: hidden [2,1024,4096], 32 q-heads / 8 kv-heads, head_dim 128,
# RoPE (neox), causal + sliding-window(512) attention, out proj.
#
# Sharding: tensor-parallel over heads across 8 cores. Core c owns q-heads
# [4c..4c+3] and kv-head c (wq cols 512c:512c+512, wk/wv cols 128c:+128).
# Each core computes its heads' attention output in TRANSPOSED layout
# [feat, tok]; per-(batch, token-half) AllGathers over the 8 cores
# concatenate the feature (partition) axis to give the full [4096, 512]
# attn output of that half on every core, and each core then applies its
# column shard of wo ([4096, 512]) to produce out[:, 512c:512c+512]. The
# host concatenates the 8 column shards.
#
# v2 schedule: a single software pipeline over the four 512-token phases
# n=0..3 (batch b=n//2, half h=n%2):
#   A(0) B(0,h0) [AG00] A(1) B(0,h1) [AG01] A(2) B(1,h0) [AG10] D(0,0)
#   A(3) B(1,h1) [AG11] D(0,1) D(1,0) D(1,1)
# Attention (phase B) is HALF-MAJOR: all 4 heads complete a 512-token half
# before any head starts the next half, so each AllGather triggers as early
# as possible and hides under >=40us of PE work. All psum tiles are single
# banks ([128,512]), which leaves room for a 3-deep score pipeline (PE
# never waits on ACT's exp).
#
# All matmuls run in bf16 (fp32 PSUM accumulation); softmax math in fp32.
#
# Layout trick: everything is computed transposed ([feature, token]) so that
# every matmul's contraction operand is already partition-major:
#   QT = wq.T @ hid     via matmul(lhsT=wq_chunk,  rhs=hidT_chunk)
#   KT = wk.T @ hid     via matmul(lhsT=wk_chunk,  rhs=hidT_chunk)
#   VT = wv.T @ hid     via matmul(lhsT=wv_chunk,  rhs=hidT_chunk)
#   V  = VT.T           via PE transposes (V needed k-major for O^T)
#   ST = K_j^T Q        via matmul(lhsT=KT_j,      rhs=QT_piece)  [k, q]
#   l  = 1^T A          via matmul(lhsT=ones_col,  rhs=at_piece)  [1, q]
#   OT = V_j^T A        via matmul(lhsT=V_j,       rhs=at_piece)  [d, q]
#   out= ag^T @ wo      via matmul(lhsT=ag_chunk,  rhs=wo_chunk)  [tok, oc]
# Softmax over k (partition axis of ST) uses exp with 0/1 post-multiplies
# for the causal diagonal / window edge, a ones-column matmul for the
# denominator, and recip + partition_broadcast + multiply to normalize.
# RoPE runs entirely on the DVE reading straight from PSUM (no ACT
# staging), so ACT only does exps and small copies.
#
# Queue discipline: DMAs traced after an AllGather trigger on the gpsimd
# queue wait for that collective, so every phase-D ag read is traced on
# gpsimd BEFORE the first trigger of any collective it must not wait for.
# hid/weight loads ride sync/scalar/vector; out writes ride sync.

import functools

import numpy as np
import ml_dtypes

BF16 = ml_dtypes.bfloat16

B, S, HID = 2, 1024, 4096
T = B * S                     # 2048 flattened tokens
NCORES = 8
D = 128                       # head dim
QH = 4                        # q heads per core
QF = QH * D                   # 512 q features per core
HC = HID // 128               # 32 hidden-dim chunks
NJ = S // 128                 # 8 k-tiles per batch
WINDOW = 512
SH = 512                      # tokens per AllGather half
OUTC = HID // NCORES          # 512 out columns per core
SCALE = D ** -0.5

# AllGather output row-block permutation: ag block ci holds the
# contribution (core, local-head) = AG_PERM[ci] (512KB inputs gather
# core-major, one chunk per core).
AG_PERM = [(ci // 4, ci % 4) for ci in range(HC)]

# Half-major score pieces. Phase B(b,h) covers q tokens
# [b*S + 512h, +512). Piece (h, j) is the part of k-tile j's 640-wide
# q-span inside this half: bank columns [c0, c0+w). Pieces with 'diag'
# carry the causal-diagonal 0/1 mask at piece cols [0,128); 'edge' pieces
# carry the window-edge mask at the last 128 cols.
def _piece(h, j):
    if h == 0:
        c0, w = 128 * j, 512 - 128 * j
        diag, edge = True, False
    elif j <= 3:
        c0, w = 0, 128 * (j + 1)
        diag, edge = False, True
    else:
        c0 = 128 * (j - 4)
        w = 512 - c0
        diag, edge = True, False
    return c0, w, diag, edge

H0_JS = [0, 1, 2, 3]              # j=0 piece is full-bank -> leads
H1_JS = [3, 4, 0, 1, 2, 5, 6, 7]  # j=3 piece is full-bank -> leads


def _build():
    import concourse.mybir as mybir
    import concourse.tile as tile
    from concourse import bacc
    from concourse.masks import make_identity

    f32, bf16 = mybir.dt.float32, mybir.dt.bfloat16
    AF = mybir.ActivationFunctionType
    ALU = mybir.AluOpType

    nc = bacc.Bacc(
        "TRN2", target_bir_lowering=False, debug=False, num_devices=NCORES
    )

    hidT = nc.dram_tensor("hidT", [128, HC * T], bf16, kind="ExternalInput")
    wq = nc.dram_tensor("wq", [128, HC * QF], bf16, kind="ExternalInput")
    wk = nc.dram_tensor("wk", [128, HC * D], bf16, kind="ExternalInput")
    wv = nc.dram_tensor("wv", [128, HC * D], bf16, kind="ExternalInput")
    wo = nc.dram_tensor("wo", [128, HC * OUTC], bf16, kind="ExternalInput")
    cosT = nc.dram_tensor("cosT", [128, T], bf16, kind="ExternalInput")
    sinT = nc.dram_tensor("sinT", [128, T], bf16, kind="ExternalInput")
    maskT = nc.dram_tensor("maskT", [128, 256], bf16, kind="ExternalInput")
    out = nc.dram_tensor("out", [T, OUTC], bf16, kind="ExternalOutput")

    with tile.TileContext(nc) as tc:
        with (
            # PSUM split by tile lifetime so ring-slot reuse can never put a
            # short-lived tile on a slot whose release is traced behind the
            # blocked PE queue: qp holds the deferred-rope q banks (released
            # mid-B), bb holds l/o accumulators, rp everything short-lived.
            tc.tile_pool(name="qp", bufs=3, space="PSUM") as qpp,
            tc.tile_pool(name="bb", bufs=2, space="PSUM") as bbp,
            tc.tile_pool(name="rp", bufs=3, space="PSUM") as rpp,
            tc.tile_pool(name="consts", bufs=1) as sbp,
            tc.tile_pool(name="hidp", bufs=27) as hidp,
            tc.tile_pool(name="ropep", bufs=2) as ropep,
            tc.tile_pool(name="attnp", bufs=11) as attnp,
            tc.tile_pool(name="miscp", bufs=2) as miscp,
            tc.tile_pool(name="agp", bufs=2) as agp,
            tc.tile_pool(name="dram", bufs=1, space="DRAM") as dramp,
        ):
            def qbank(name):
                return qpp.tile([128, 512], f32, tag="qb", bufs=3, name=name)

            def bbank(name):
                return bbp.tile([128, 512], f32, tag="bb", bufs=2, name=name)

            def rbank(name):
                return rpp.tile([128, 512], f32, tag="rb", bufs=3, name=name)

            def load_split(dst, src, ncols, pieces, skip_first=0, eng=None):
                eng = eng or nc.sync
                step = ncols // pieces
                for i in range(skip_first, pieces):
                    eng.dma_start(
                        dst[:, i * step:(i + 1) * step],
                        src[:, i * step:(i + 1) * step],
                    )

            hidT_r = hidT[:, :].rearrange("p (h t) -> p h t", h=HC)

            def load_hid(n, h2, eng):
                # one DMA covers h-chunks 2*h2 and 2*h2+1
                ht = hidp.tile([128, 1024], bf16, tag="hid", bufs=27,
                               name=f"ht{n}_{h2}")
                eng.dma_start(
                    ht[:].rearrange("p (a t) -> p a t", a=2),
                    hidT_r[:, 2 * h2:2 * h2 + 2, n * 512:(n + 1) * 512],
                )
                return ht

            def prefetch_hid(n, eng, eng2=None):
                # eng2 (if given) takes odd tiles so two queues share the
                # startup issue latency
                return [
                    load_hid(n, h2, eng2 if (eng2 and h2 % 2) else eng)
                    for h2 in range(16)
                ]

            # startup-critical loads first, interleaved in consumption order
            # across sync (even wq pieces + wk/wv) and scalar (hid evens +
            # odd wq pieces); hid odds ride gpsimd. Group g of phase A needs
            # wq pieces 2g/2g+1 and hid tiles 2g/2g+1.
            wq_sb = sbp.tile([128, HC * QF], bf16, name="wq_sb")
            wk_sb = sbp.tile([128, HC * D], bf16, name="wk_sb")
            wv_sb = sbp.tile([128, HC * D], bf16, name="wv_sb")

            def wq_piece(i, eng):
                eng.dma_start(wq_sb[:, i * 1024:(i + 1) * 1024],
                              wq[:, i * 1024:(i + 1) * 1024])

            wq_piece(0, nc.sync)
            wq_piece(1, nc.sync)
            nc.sync.dma_start(wk_sb[:, 0:1024], wk[:, 0:1024])
            nc.sync.dma_start(wv_sb[:, 0:1024], wv[:, 0:1024])
            pre0 = []
            for h2 in range(16):
                if h2 % 2 == 0:
                    pre0.append(load_hid(0, h2, nc.scalar))
                    if h2 >= 2:
                        wq_piece(h2, nc.sync)
                        wq_piece(h2 + 1, nc.scalar)
                else:
                    pre0.append(load_hid(0, h2, nc.gpsimd))
            load_split(wk_sb, wk, HC * D, 4, skip_first=1)
            load_split(wv_sb, wv, HC * D, 4, skip_first=1)
            cos_sb = sbp.tile([128, T], bf16, name="cos_sb")
            sin_sb = sbp.tile([128, T], bf16, name="sin_sb")

            # per-batch transposed activations (region-reused across batches)
            QT_sb = sbp.tile([128, QH * S], bf16, name="QT_sb")
            KT_sb = sbp.tile([128, S], bf16, name="KT_sb")
            VT_sb = sbp.tile([128, S], bf16, name="VT_sb")
            V_sb = sbp.tile([128, S], bf16, name="V_sb")

            # small constants for B (tiny DMAs / on-chip init)
            mask_sb = sbp.tile([128, 256], bf16, name="mask_sb")
            nc.sync.dma_start(mask_sb[:], maskT[:, :])
            ones_sb = sbp.tile([128, 1], bf16, name="ones_sb")
            nc.vector.memset(ones_sb[:], 1.0)
            ident_sb = sbp.tile([128, 128], bf16, name="ident_sb")
            make_identity(nc, ident_sb[:])
            wo_sb = sbp.tile([128, HC * OUTC], bf16, name="wo_sb")

            # per-(batch, half) AllGathers
            attn_local = [
                [dramp.tile([QF, SH], bf16, name=f"attn_local{b}_{h}")
                 for h in range(2)]
                for b in range(B)
            ]
            ag_out = [
                [dramp.tile([HID, SH], bf16, name=f"ag_out{b}_{h}",
                            addr_space="Shared")
                 for h in range(2)]
                for b in range(B)
            ]

            # tiny dummy AllGather (fired right after the startup gpsimd
            # DMAs): absorbs the first collective's ring-warmup (~30us)
            # under phase A(0)'s compute
            warm_in = dramp.tile([128, 16], bf16, name="warm_in")
            warm_out = dramp.tile([NCORES * 128, 16], bf16, name="warm_out",
                                  addr_space="Shared")

            def warmup_ag():
                nc.gpsimd.dma_start(warm_in[:, :], mask_sb[:, 0:16])
                nc.gpsimd.collective_compute(
                    "AllGather",
                    mybir.AluOpType.bypass,
                    ins=[warm_in[:, :]],
                    outs=[warm_out[:, :]],
                    replica_groups=[list(range(NCORES))],
                )

            def load_trig(n):
                nc.gpsimd.dma_start(
                    cos_sb[:, n * 512:(n + 1) * 512],
                    cosT[:, n * 512:(n + 1) * 512],
                )
                nc.gpsimd.dma_start(
                    sin_sb[:, n * 512:(n + 1) * 512],
                    sinT[:, n * 512:(n + 1) * 512],
                )

            def rope_dve(dst, src_ps, n, tag):
                # neox rotate-half straight from psum on the DVE:
                #   dst = x*cos + swap_halves(x)*sin_signed
                # (sin rows 0:64 arrive pre-negated from the host)
                c = cos_sb[:, n * 512:(n + 1) * 512]
                sg = sin_sb[:, n * 512:(n + 1) * 512]
                t1 = ropep.tile([128, 512], f32, tag="rt1", name=f"t1{tag}")
                t2 = ropep.tile([128, 512], f32, tag="rt2", name=f"t2{tag}")
                nc.vector.tensor_tensor(t1[:], src_ps, c, ALU.mult)
                nc.vector.tensor_tensor(
                    t2[0:64, :], src_ps[64:128, :], sg[0:64, :], ALU.mult
                )
                nc.vector.tensor_tensor(
                    t2[64:128, :], src_ps[0:64, :], sg[64:128, :], ALU.mult
                )
                nc.vector.tensor_tensor(dst, t1[:], t2[:], ALU.add)

            # ---- phase A(n): projections, transposed, weight-stationary ----
            # Q ropes for heads 1..3 are deferred into phase B (traced just
            # before each head's scores) so B's mask multiplies don't queue
            # on the DVE behind ropes that aren't needed yet.
            def phase_a(n, pre):
                tok0 = (n % 2) * 512
                k_ps = rbank(f"k{n}")
                v_ps = rbank(f"v{n}")
                q_ps = [rbank(f"q{n}_0")] + [
                    qbank(f"q{n}_{m}") for m in range(1, QH)
                ]
                for hg in range(0, HC, 4):
                    hts = [
                        pre[hg // 2 + k // 2][:, (k % 2) * 512:(k % 2) * 512 + 512]
                        for k in range(4)
                    ]
                    def kv_mms():
                        for k, h in enumerate(range(hg, hg + 4)):
                            nc.tensor.matmul(
                                k_ps[:], wk_sb[:, h * 128:(h + 1) * 128],
                                hts[k], start=(h == 0), stop=(h == HC - 1),
                            )
                        for k, h in enumerate(range(hg, hg + 4)):
                            nc.tensor.matmul(
                                v_ps[:], wv_sb[:, h * 128:(h + 1) * 128],
                                hts[k], start=(h == 0), stop=(h == HC - 1),
                            )
                    def q_mms():
                        for m in range(QH):
                            for k, h in enumerate(range(hg, hg + 4)):
                                nc.tensor.matmul(
                                    q_ps[m][:],
                                    wq_sb[:, (h * QH + m) * 128:(h * QH + m + 1) * 128],
                                    hts[k], start=(h == 0), stop=(h == HC - 1),
                                )
                    if hg == HC - 4:
                        # kv stop early so the K rope overlaps the tail q mms
                        kv_mms()
                        q_mms()
                    else:
                        q_mms()
                        kv_mms()
                rope_dve(KT_sb[:, tok0:tok0 + 512], k_ps[:], n, f"K{n}")
                nc.scalar.copy(VT_sb[:, tok0:tok0 + 512], v_ps[:])
                rope_dve(
                    QT_sb[:, 0 * S + tok0:0 * S + tok0 + 512],
                    q_ps[0][:], n, f"q{n}_0",
                )
                return q_ps

            # ---- V = VT.T via PE transposes (pads the A->B rope latency) ----
            def v_transpose(n):
                for tt in range(4 * (n % 2), 4 * (n % 2) + 4):
                    trp = rpp.tile([128, 128], bf16, tag="rb", bufs=3,
                                   name=f"tr{n}_{tt}")
                    nc.tensor.transpose(
                        trp[:], VT_sb[:, tt * 128:(tt + 1) * 128],
                        ident_sb[:],
                    )
                    nc.vector.tensor_copy(V_sb[:, tt * 128:(tt + 1) * 128],
                                          trp[:])

            # ---- phase B(b,h): half-major windowed attention ----
            def phase_b_half(b, h, q_ps):
                n = 2 * b + h
                tok0 = h * 512
                js = H0_JS if h == 0 else H1_JS
                for m in range(QH):
                    if m > 0:
                        rope_dve(
                            QT_sb[:, m * S + tok0:m * S + tok0 + 512],
                            q_ps[m][:], n, f"q{n}_{m}",
                        )
                    l_ps = bbank(f"l{b}{h}{m}")
                    o_ps = bbank(f"o{b}{h}{m}")
                    at_tiles = {}

                    def scores(j):
                        c0, w, diag, edge = _piece(h, j)
                        sc = rbank(f"sc{b}{h}{m}{j}")
                        kslice = KT_sb[:, j * 128:(j + 1) * 128]
                        q0 = m * S + h * 512 + c0
                        nc.tensor.matmul(
                            sc[:, 0:w], kslice, QT_sb[:, q0:q0 + w],
                            start=True, stop=True,
                        )
                        at = attnp.tile([128, 512], bf16, tag="attn", bufs=11,
                                        name=f"at{b}{h}{m}{j}")
                        nc.scalar.activation(at[:, 0:w], sc[:, 0:w], AF.Exp)
                        if diag:
                            nc.vector.tensor_tensor(
                                at[:, 0:128], at[:, 0:128],
                                mask_sb[:, 0:128], ALU.mult,
                            )
                        if edge:
                            nc.vector.tensor_tensor(
                                at[:, w - 128:w], at[:, w - 128:w],
                                mask_sb[:, 128:256], ALU.mult,
                            )
                        at_tiles[j] = at

                    def acc(j, which, first, final):
                        c0, w, _, _ = _piece(h, j)
                        at = at_tiles[j]
                        if which == "l":
                            nc.tensor.matmul(
                                l_ps[0:1, c0:c0 + w], ones_sb[:],
                                at[:, 0:w], start=first, stop=final,
                            )
                        else:
                            vslice = V_sb[:, j * 128:(j + 1) * 128]
                            nc.tensor.matmul(
                                o_ps[:, c0:c0 + w], vslice,
                                at[:, 0:w], start=first, stop=final,
                            )

                    # all l accs run before the o accs so l stops early and
                    # the norm chain (lsb/recip/bcast) overlaps the o pass,
                    # putting the oT write (and the AG trigger it feeds)
                    # right behind the last o matmul
                    LOOK = 3
                    for i in range(min(LOOK, len(js))):
                        scores(js[i])
                    for i, j in enumerate(js):
                        if i + LOOK < len(js):
                            scores(js[i + LOOK])
                        acc(j, "l", i == 0, i == len(js) - 1)
                    for i, j in enumerate(js):
                        acc(j, "o", i == 0, i == len(js) - 1)

                    # normalize: oT = o_ps * bcast(1/l), then stage to DRAM
                    l_sb = miscp.tile([1, 512], f32, tag="lsb",
                                      name=f"l_sb{b}{h}{m}")
                    nc.scalar.copy(l_sb[:], l_ps[0:1, :])
                    lrec = miscp.tile([1, 512], f32, tag="lrec",
                                      name=f"lrec{b}{h}{m}")
                    nc.vector.reciprocal_approx_fast(lrec[:], l_sb[:])
                    bcr = miscp.tile([128, 512], f32, tag="bcr",
                                     name=f"bcr{b}{h}{m}")
                    nc.gpsimd.partition_broadcast(bcr[:], lrec[:])
                    oT = miscp.tile([128, 512], bf16, tag="osb",
                                    name=f"oT{b}{h}{m}")
                    nc.vector.tensor_tensor(oT[:], o_ps[:], bcr[:], ALU.mult)
                    # MUST NOT ride gpsimd: a gpsimd DMA traced after an AG
                    # trigger waits for that collective, and a blocked oT
                    # write stalls the whole norm pipeline behind it
                    nc.sync.dma_start(
                        attn_local[b][h][m * 128:(m + 1) * 128, :], oT[:],
                    )

            def all_gather(b, h):
                nc.gpsimd.collective_compute(
                    "AllGather",
                    ALU.bypass,
                    ins=[attn_local[b][h][:, :]],
                    outs=[ag_out[b][h][:, :]],
                    replica_groups=[list(range(NCORES))],
                )

            # ---- phase D: out projection on this core's wo column shard ----
            # ag reads are traced on gpsimd BEFORE the trigger of any AG they
            # must not wait on (a gpsimd DMA traced after a trigger waits for
            # that collective to complete).
            def phase_d_reads(b, pp):
                # 4 big reads (8 row-blocks each) keep the gpsimd issue
                # count low so a following AG trigger isn't delayed
                ag_ts = []
                for afg in range(0, HC, 8):
                    ag_t = agp.tile([128, 4096], bf16, tag="ag", bufs=2,
                                    name=f"ag{b}_{pp}_{afg}")
                    nc.gpsimd.dma_start(
                        ag_t[:].rearrange("p (a t) -> p a t", a=8),
                        ag_out[b][pp][afg * 128:(afg + 8) * 128, :]
                        .rearrange("(a p) t -> p a t", a=8),
                    )
                    ag_ts.append(ag_t)
                return ag_ts

            def phase_d(b, pp, ag_ts):
                # D never overlaps B, so its 4 accumulators borrow the
                # qp (3) + bb (1) banks
                ops = [qbank(f"op{b}_{pp}_{q}") for q in range(3)]
                ops.append(bbank(f"op{b}_{pp}_3"))
                for gi, afg in enumerate(range(0, HC, 4)):
                    ag_t = ag_ts[gi // 2]
                    c0 = (gi % 2) * 2048
                    for tt in range(4):
                        for k, af in enumerate(range(afg, afg + 4)):
                            nc.tensor.matmul(
                                ops[tt][:],
                                ag_t[:, c0 + k * 512 + tt * 128:
                                     c0 + k * 512 + (tt + 1) * 128],
                                wo_sb[:, af * OUTC:(af + 1) * OUTC],
                                start=(af == 0), stop=(af == HC - 1),
                            )
                for q in range(4):
                    ob = miscp.tile([128, 512], bf16, tag="ob",
                                    name=f"ob{b}_{pp}_{q}")
                    # drain psum on alternating engines
                    if q % 2 == 0:
                        nc.vector.tensor_copy(ob[:], ops[q][:])
                    else:
                        nc.scalar.copy(ob[:], ops[q][:])
                    r0 = b * S + pp * 512 + q * 128
                    nc.sync.dma_start(out[r0:r0 + 128, :], ob[:])

            # ---- orchestration ----
            # hid prefetch and wo ride the (otherwise idle) sync queue so
            # the ACT queue stays free for B's exps. Phase-D ag reads also
            # ride sync: the tile framework wires the collective-completion
            # dependency, and the sync queue carries nothing that a blocked
            # read could starve.
            load_trig(0)
            load_trig(1)
            warmup_ag()
            qp0 = phase_a(0, pre0)
            pre1 = prefetch_hid(1, nc.sync)
            load_split(wo_sb, wo, HC * OUTC, 16, eng=nc.sync)
            v_transpose(0)
            phase_b_half(0, 0, qp0)
            all_gather(0, 0)
            qp1 = phase_a(1, pre1)
            pre2 = prefetch_hid(2, nc.sync)
            load_trig(2)
            v_transpose(1)
            phase_b_half(0, 1, qp1)
            all_gather(0, 1)
            qp2 = phase_a(2, pre2)
            pre3 = prefetch_hid(3, nc.sync)
            load_trig(3)
            v_transpose(2)
            phase_b_half(1, 0, qp2)
            ag00_ts = phase_d_reads(0, 0)   # gated on AG00/AG01 (both done)
            all_gather(1, 0)
            phase_d(0, 0, ag00_ts)
            qp3 = phase_a(3, pre3)
            v_transpose(3)
            phase_b_half(1, 1, qp3)
            ag01_ts = phase_d_reads(0, 1)   # gated on AG01 (done)
            ag10_ts = phase_d_reads(1, 0)   # gated on AG10 (done)
            all_gather(1, 1)
            phase_d(0, 1, ag01_ts)
            phase_d(1, 0, ag10_ts)
            ag11_ts = phase_d_reads(1, 1)   # genuinely waits on AG11
            phase_d(1, 1, ag11_ts)

    nc.compile()
    return nc


@functools.lru_cache(maxsize=1)
def _get_nc():
    return _build()


def _prep_in_maps(hidden_states, wq, wk, wv, wo, cos, sin):
    hs = np.ascontiguousarray(np.asarray(hidden_states, np.float32)).reshape(T, HID)
    hidT = hs.T.reshape(HC, 128, T).transpose(1, 0, 2).reshape(128, HC * T)
    hidT = np.ascontiguousarray(hidT).astype(BF16)

    wq = np.asarray(wq, np.float32) * SCALE
    wk = np.asarray(wk, np.float32)
    wv = np.asarray(wv, np.float32)
    wo = np.asarray(wo, np.float32)

    cosT = np.ascontiguousarray(np.asarray(cos, np.float32).T)  # [64, S]
    sinT = np.ascontiguousarray(np.asarray(sin, np.float32).T)
    cosT2 = np.concatenate([cosT, cosT], axis=1)   # [64, T]
    sinT2 = np.concatenate([sinT, sinT], axis=1)
    cos128 = np.concatenate([cosT2, cosT2], axis=0).astype(BF16)  # [128, T]
    sin128 = np.concatenate([-sinT2, sinT2], axis=0).astype(BF16)

    r = np.arange(128)[:, None]
    c = np.arange(128)[None, :]
    SL = np.where(c < r, 0.0, 1.0)  # diag tile: invalid where q < k
    SU = np.where(c > r, 0.0, 1.0)  # window-edge tile: invalid where q-k > W
    maskadd = np.concatenate([SL, SU], axis=1).astype(BF16)

    def shard_w(w, cols, core):
        ws = w[:, core * cols:(core + 1) * cols]
        return np.ascontiguousarray(
            ws.reshape(HC, 128, cols).transpose(1, 0, 2).reshape(128, HC * cols)
        ).astype(BF16)

    def shard_wo(w, core):
        ws = w[:, core * OUTC:(core + 1) * OUTC]
        blocks = []
        for ci in range(HC):
            c2, hp = AG_PERM[ci]
            g = 4 * c2 + hp
            blocks.append(ws[g * 128:(g + 1) * 128, :])
        arr = np.stack(blocks, 0)
        return np.ascontiguousarray(
            arr.transpose(1, 0, 2).reshape(128, HC * OUTC)
        ).astype(BF16)

    in_maps = []
    for cidx in range(NCORES):
        in_maps.append({
            "hidT": hidT,
            "wq": shard_w(wq, QF, cidx),
            "wk": shard_w(wk, D, cidx),
            "wv": shard_w(wv, D, cidx),
            "wo": shard_wo(wo, cidx),
            "cosT": cos128,
            "sinT": sin128,
            "maskT": maskadd,
        })
    return in_maps


def run(inputs, trace=False, **spmd_kwargs):
    from concourse.bass_utils import run_bass_kernel_spmd

    window = int(np.asarray(inputs["window"]))
    assert window == WINDOW, f"kernel compiled for window={WINDOW}, got {window}"
    nc = _get_nc()
    in_maps = _prep_in_maps(
        inputs["hidden_states"], inputs["wq"], inputs["wk"], inputs["wv"],
        inputs["wo"], inputs["cos"], inputs["sin"],
    )
    res = run_bass_kernel_spmd(
        nc, in_maps, list(range(NCORES)), trace=trace, **spmd_kwargs
    )
    parts = [np.asarray(res.results[i]["out"], np.float32) for i in range(NCORES)]
    full = np.concatenate(parts, axis=1).reshape(B, S, HID)
    return full, res


def kernel(**inputs):
    return run(inputs, trace=False)[0]
